# revision 16
# baseline (speedup 1.0000x reference)
"""DrBCNet GNN message-passing kernel for 8 Trainium2 NeuronCores — v5.

Strategy (dst-sharded, sparse gather + selector matmuls, 3-chunk pipeline):
  - Nodes globally degree-sorted, dealt round-robin to 8 cores (3750 each);
    GRU/norm/decoder node-local in column layout (hT [feat, pos]).
  - Node positions split into 3 chunks A|B|C (bank-aligned). The h row-table
    (fp8 pairs, [30720 rows, 128B]) lives in DRAM (Shared), rebuilt each
    layer by THREE chunk AllGathers (A, B, C) so collectives pipeline with
    the gather stream.
  - Edges are grouped by (src chunk = phase, dst bank): phase-p gathers read
    only the chunk-p row range of the table (chunk-relative indices), so
    they start as soon as AG_p lands while later AGs are still in flight.
    Per (phase, bank) partial aggregates accumulate via selector matmuls in
    PSUM, evicted into an SBUF f32 accumulator; after the last phase the
    bank's aggregate feeds the GRU. GRU/norm/AG(l+1) run chunk-by-chunk so
    AG_A(l+1) fires while chunk B/C of layer l still compute.
  - GRU: bf16 weight lhsT, i+h gate sums in PSUM, biases folded into the
    PSUM-evacuating activations; Sqrt (l2norm) phase-batched per chunk.
"""

import functools
import os

import numpy as np

CORES = 8
H = 128
L = 5
BANK = 512
NORM_EPS_SQ = 1e-24
WIN_EDGES = int(os.environ.get("WE", "512"))
TPW = WIN_EDGES // 128
HALF_W = WIN_EDGES // 2
# chunk A = leading A_COLS cols; B = [A_COLS, CHB*512); C = rest
A_COLS = int(os.environ.get("ACOLS", "256"))
CHB = int(os.environ.get("CHB", "5"))
CALL_WINDOWS = int(os.environ.get("CW", "4"))  # windows per dma_gather call
# processing order of chunks (chain + phase stream): small A first, big B last
CH_ORDER = [int(x) for x in os.environ.get("CHORD", "0,2,1").split(",")]


# ---------------------------------------------------------------- host planning
def _plan(edge_src, edge_dst, n_nodes):
    npc = n_nodes // CORES
    npc_pad = ((npc + 127) // 128) * 128
    ntiles = npc_pad // 128

    deg = np.bincount(edge_dst, minlength=n_nodes)
    gorder = np.argsort(-deg, kind="stable")
    gpos = np.empty(n_nodes, np.int64)
    gpos[gorder] = np.arange(n_nodes)
    owner = gpos % CORES
    pos = gpos // CORES
    order_per_core = [gorder[r::CORES] for r in range(CORES)]

    # chunks in per-core position space (128-aligned => even sizes)
    cb = [0, A_COLS, CHB * BANK, npc_pad]
    csizes = [cb[1] - cb[0], cb[2] - cb[1], cb[3] - cb[2]]
    # global table row base of each chunk (chunk-major layout)
    cbases = [0, CORES * csizes[0], CORES * (csizes[0] + csizes[1])]
    # bank col ranges: chunk A is one narrow bank, then pad to the B|C
    # boundary, then 512-wide banks
    bank_edges = [0, A_COLS]
    x = A_COLS
    while x < cb[2]:
        step = min(BANK, cb[2] - x)
        x += step
        bank_edges.append(x)
    while x < npc:
        step = min(BANK, npc - x)
        x += step
        bank_edges.append(x)
    banks = [
        (bank_edges[i], min(bank_edges[i + 1], npc) - bank_edges[i])
        for i in range(len(bank_edges) - 1)
    ]
    n_banks = len(banks)
    first_c_bank = next(i for i, (s0, _) in enumerate(banks) if s0 >= cb[2])
    chunk_banks = [
        [0],
        list(range(1, first_c_bank)),
        list(range(first_c_bank, n_banks)),
    ]

    cidx = np.where(pos < cb[1], 0, np.where(pos < cb[2], 1, 2))
    base = np.take(cbases, cidx)
    size = np.take(csizes, cidx)
    start = np.take(np.array(cb[:3]), cidx)
    tpos = base + owner * size + (pos - start)

    srcrow_all = tpos[edge_src]
    src_phase = cidx[edge_src]
    dpos = pos[edge_dst]
    down = owner[edge_dst]

    # per-core per-column counts split by (phase, parity of table row)
    tpar = srcrow_all % 2
    Cnt = np.zeros((3, 2, CORES, npc), np.int64)
    for r in range(CORES):
        m = down == r
        for p in range(3):
            for par in range(2):
                np.add.at(
                    Cnt[p, par, r], dpos[m & (src_phase == p) & (tpar == par)], 1
                )

    # windows per (phase, bank): greedy pack columns while per-core per-parity
    # count fits HALF_W. Stream order: phase in CH_ORDER, dst chunk in
    # CH_ORDER, bank ascending within chunk.
    windows = []  # list of (phase, bank, col_a, col_b)
    for p in CH_ORDER:
        for b in [bb for c in CH_ORDER for bb in chunk_banks[c]]:
            c0 = banks[b][0]
            c1 = c0 + banks[b][1]
            w0 = c0
            rune = np.zeros(CORES, np.int64)
            runo = np.zeros(CORES, np.int64)
            for q in range(c0, c1):
                ce = Cnt[p, 0, :, q]
                co = Cnt[p, 1, :, q]
                if (rune + ce).max() > HALF_W or (runo + co).max() > HALF_W:
                    windows.append((p, b, w0, q))
                    w0 = q
                    rune = ce.copy()
                    runo = co.copy()
                else:
                    rune += ce
                    runo += co
            windows.append((p, b, w0, c1))
    nwin = len(windows)
    win_width = [e - a for (_, _, a, e) in windows]
    sel_off = np.concatenate([[0], np.cumsum([TPW * w for w in win_width])])
    total_sel = int(sel_off[-1])
    total_idx = nwin * WIN_EDGES

    import ml_dtypes

    # per-core idx + selector maps. tiles 0..TPW/2-1 of each window: even
    # parity srcs; rest: odd. idx is the CHUNK-RELATIVE pair row
    # ((tpos - cbases[p]) // 2) so gathers can bind to the chunk row range.
    idx_maps, sel_maps = [], []
    for r in range(CORES):
        eidx = np.nonzero(down == r)[0]
        dp = dpos[eidx]
        sr = srcrow_all[eidx]
        ph = src_phase[eidx]
        key = ph * (npc * 2) + dp * 2 + (sr % 2)
        o = np.argsort(key, kind="stable")
        dp, sr, ph = dp[o], sr[o], ph[o]
        par = sr % 2
        skey = ph * (npc * 2) + dp * 2 + par  # sorted within each phase
        idxs = np.zeros(total_idx, np.int16)
        sel = np.zeros((128, total_sel), np.float32)
        for w, (p, b, a, e) in enumerate(windows):
            S = e - a
            base_i = w * WIN_EDGES
            lo = np.searchsorted(skey, p * (npc * 2) + a * 2)
            hi = np.searchsorted(skey, p * (npc * 2) + e * 2)
            seg = slice(lo, hi)
            pvals = par[seg]
            for pp in (0, 1):
                m = np.nonzero(pvals == pp)[0]
                cnt = len(m)
                assert cnt <= HALF_W, (r, w, pp, cnt)
                slot0 = base_i + pp * HALF_W
                rows_sr = sr[seg][m]
                idxs[slot0: slot0 + cnt] = (
                    (rows_sr - cbases[p]) // 2
                ).astype(np.int16)
                loc = (dp[seg][m] - a).astype(np.int64)
                j = np.arange(cnt) // 128 + (TPW // 2) * pp
                q = np.arange(cnt) % 128
                sel[q, sel_off[w] + j * S + loc] = 1.0
        idx_w = np.zeros((128, total_idx // 16), np.int16)
        wrapped = idxs.reshape(total_idx // 16, 16).T
        for g in range(8):
            idx_w[g * 16: (g + 1) * 16, :] = wrapped
        idx_maps.append(idx_w)
        sel_maps.append(sel.astype(ml_dtypes.float8_e4m3fn))

    # window index ranges per (phase, bank) and per phase
    pb_wins = {}
    phase_wins = {}
    for w, (p, b, a, e) in enumerate(windows):
        if (p, b) not in pb_wins:
            pb_wins[(p, b)] = [w, w]
        pb_wins[(p, b)][1] = w
        if p not in phase_wins:
            phase_wins[p] = [w, w]
        phase_wins[p][1] = w

    return dict(
        npc=npc,
        npc_pad=npc_pad,
        ntiles=ntiles,
        n_banks=n_banks,
        banks=banks,
        cb=cb,
        csizes=csizes,
        cbases=cbases,
        chunk_banks=chunk_banks,
        nwin=nwin,
        windows=windows,
        sel_off=sel_off,
        total_sel=total_sel,
        total_idx=total_idx,
        pb_wins=pb_wins,
        phase_wins=phase_wins,
        order_per_core=order_per_core,
        idx_maps=idx_maps,
        sel_maps=sel_maps,
    )


# ---------------------------------------------------------------- bass program
def _build(meta):
    import concourse.bacc as bacc
    import concourse.mybir as mybir
    import concourse.tile as tile
    from concourse import library_config

    npc = meta["npc"]
    npc_pad = meta["npc_pad"]
    ntiles = meta["ntiles"]
    n_banks = meta["n_banks"]
    banks = meta["banks"]
    cb = meta["cb"]
    csizes = meta["csizes"]
    cbases = meta["cbases"]
    chunk_banks = meta["chunk_banks"]
    windows = meta["windows"]
    sel_off = meta["sel_off"]
    total_sel = meta["total_sel"]
    total_idx = meta["total_idx"]
    pb_wins = meta["pb_wins"]
    phase_wins = meta["phase_wins"]
    n_tbl = CORES * npc_pad
    f32 = mybir.dt.float32
    bf16 = mybir.dt.bfloat16
    i16 = mybir.dt.int16
    fp8 = mybir.dt.float8e4
    AF = mybir.ActivationFunctionType
    OP = mybir.AluOpType

    nc = bacc.Bacc(
        "TRN2",
        target_bir_lowering=False,
        debug=False,
        num_devices=CORES,
        dynamic_dma_scratch_size=int(os.environ.get("SCR", "49152")),
    )

    # I/O
    xT_d = nc.dram_tensor("xT", [3, npc], f32, kind="ExternalInput")
    idx_d = nc.dram_tensor("idx", [128, total_idx // 16], i16, kind="ExternalInput")
    sel_d = nc.dram_tensor("sel", [128, total_sel], fp8, kind="ExternalInput")
    w1T_d = nc.dram_tensor("w1T", [3, 128], f32, kind="ExternalInput")
    b1_d = nc.dram_tensor("b1", [128, 1], f32, kind="ExternalInput")
    wihT_d = nc.dram_tensor("wihT", [128, 3 * H], bf16, kind="ExternalInput")
    whhT_d = nc.dram_tensor("whhT", [128, 3 * H], bf16, kind="ExternalInput")
    brz_d = nc.dram_tensor("brz", [128, 4], f32, kind="ExternalInput")  # br,bz,bin,bhn
    w2T_d = nc.dram_tensor("w2T", [128, 128], f32, kind="ExternalInput")
    b2_d = nc.dram_tensor("b2", [1, 128], f32, kind="ExternalInput")
    binrow_d = nc.dram_tensor("binrow", [1, 128], bf16, kind="ExternalInput")
    bhnrow_d = nc.dram_tensor("bhnrow", [1, 128], bf16, kind="ExternalInput")
    out_d = nc.dram_tensor("out", [npc_pad, 128], f32, kind="ExternalOutput")

    ag_in = [
        nc.dram_tensor(f"agin{l}", [npc_pad // 2, 2, 128], fp8) for l in range(L)
    ]
    tables = [
        nc.dram_tensor(f"table{l}", [n_tbl // 2, 256], fp8, addr_space="Shared")
        for l in range(L)
    ]
    groups = [list(range(CORES))]

    with tile.TileContext(nc) as tc:
        import contextlib

        stack = contextlib.ExitStack()
        nc.gpsimd.load_library(library_config.mlp)
        per = stack.enter_context(tc.tile_pool(name="per", bufs=1))

        def _T(shape, dtype, name=None):
            return per.tile(shape, dtype, name=name, tag=name)

        idx_sb = _T([128, total_idx // 16], i16, name="idx_sb")
        sel_sb = _T([128, total_sel], fp8, name="sel_sb")
        hT = _T([128, npc], f32, name="hT")
        hmaxT = _T([128, npc], f32, name="hmaxT")
        hT16 = _T([128, npc], bf16, name="hT16")
        aggF = _T([128, npc], f32, name="aggF")
        w1T_sb = _T([3, 128], f32, name="w1T_sb")
        b1_sb = _T([128, 1], f32, name="b1_sb")
        wihT_sb = _T([128, 3 * H], bf16, name="wihT_sb")
        whhT_sb = _T([128, 3 * H], bf16, name="whhT_sb")
        brz_sb = _T([128, 4], f32, name="brz_sb")
        w2T_sb = _T([128, 128], f32, name="w2T_sb")
        b2_sb = _T([1, 128], f32, name="b2_sb")
        ones_col = _T([128, 1], f32, name="ones_col")
        ones_row = _T([1, BANK], bf16, name="ones_row")
        binrow = _T([1, 128], bf16, name="binrow")
        bhnrow = _T([1, 128], bf16, name="bhnrow")
        onesk1 = _T([1, 128], f32, name="onesk1")
        eps_sb = _T([1, 1], f32, name="eps_sb")

        gpool = stack.enter_context(
            tc.tile_pool(name="gpool", bufs=int(os.environ.get("GB", "2")))
        )
        epool = stack.enter_context(
            tc.tile_pool(name="epool", bufs=int(os.environ.get("EB", "4")))
        )
        xpool = stack.enter_context(tc.tile_pool(name="xpool", bufs=2))
        tpool = stack.enter_context(
            tc.tile_pool(name="tpool", bufs=int(os.environ.get("TB", "2")))
        )
        ps = stack.enter_context(tc.tile_pool(name="ps", bufs=8, space="PSUM"))

        nc.sync.dma_start(out=aggF[0:3, :], in_=xT_d[:])
        nc.sync.dma_start(out=idx_sb[:], in_=idx_d[:])
        nc.sync.dma_start(out=w1T_sb[:], in_=w1T_d[:])
        nc.sync.dma_start(out=b1_sb[:], in_=b1_d[:])
        nc.sync.dma_start(out=wihT_sb[:], in_=wihT_d[:])
        nc.sync.dma_start(out=whhT_sb[:], in_=whhT_d[:])
        nc.sync.dma_start(out=brz_sb[:], in_=brz_d[:])
        nc.sync.dma_start(out=w2T_sb[:], in_=w2T_d[:])
        nc.sync.dma_start(out=b2_sb[:], in_=b2_d[:])
        nc.vector.memset(eps_sb[:], NORM_EPS_SQ)
        nc.vector.memset(ones_col[:], 1.0)
        nc.vector.memset(ones_row[:], 1.0)
        nc.sync.dma_start(out=binrow[:], in_=binrow_d[:])
        nc.sync.dma_start(out=bhnrow[:], in_=bhnrow_d[:])
        nc.vector.memset(onesk1[:], 1.0)
        nc.scalar.dma_start(out=sel_sb[:], in_=sel_d[:])

        def norm_phase(bank_list, h8=None, h8_base=0):
            """l2norm hT strips for several banks; single Sqrt table window.
            If h8 is given, also emit the fp8 column copy (staging input)
            right after each bank's hT update."""
            ns_list = []
            for b in bank_list:
                s0, w = banks[b]
                sq = tpool.tile([128, BANK], f32, tag="sq", name=f"sq{b}")
                nc.vector.tensor_tensor(
                    out=sq[:, :w], in0=hT[:, s0: s0 + w], in1=hT[:, s0: s0 + w],
                    op=OP.mult,
                )
                ns_ps = ps.tile([1, BANK], f32, tag="ps", name=f"ns{b}")
                nc.tensor.matmul(
                    out=ns_ps[:1, :w], lhsT=ones_col[:], rhs=sq[:, :w],
                    start=True, stop=True,
                )
                ns_list.append(ns_ps)
            inv_list = []
            for b, ns_ps in zip(bank_list, ns_list):
                s0, w = banks[b]
                srt = tpool.tile([1, BANK], f32, tag="srt", name=f"srt{b}")
                nc.scalar.activation(
                    out=srt[:1, :w], in_=ns_ps[:1, :w], func=AF.Sqrt,
                    bias=eps_sb[:1, :1],
                )
                inv_t = tpool.tile([1, BANK], f32, tag="inv_t", name=f"inv{b}")
                nc.vector.reciprocal(out=inv_t[:1, :w], in_=srt[:1, :w])
                inv_list.append(inv_t)
            for b, inv_t in zip(bank_list, inv_list):
                s0, w = banks[b]
                bc_ps = ps.tile([128, BANK], f32, tag="ps", name=f"bc{b}")
                nc.tensor.matmul(
                    out=bc_ps[:, :w], lhsT=onesk1[:1, :], rhs=inv_t[:1, :w],
                    start=True, stop=True,
                )
                nc.vector.tensor_tensor(
                    out=hT[:, s0: s0 + w], in0=hT[:, s0: s0 + w],
                    in1=bc_ps[:, :w], op=OP.mult,
                )
                nc.vector.tensor_copy(
                    out=hT16[:, s0: s0 + w], in_=hT[:, s0: s0 + w]
                )
                if h8 is not None:
                    nc.scalar.activation(
                        out=h8[:, s0 - h8_base: s0 - h8_base + w],
                        in_=hT[:, s0: s0 + w], func=AF.Copy,
                    )
                nc.vector.tensor_tensor(
                    out=hmaxT[:, s0: s0 + w], in0=hmaxT[:, s0: s0 + w],
                    in1=hT[:, s0: s0 + w], op=OP.max,
                )

        def gru_bank(l, b, aggS):
            """GRU for bank b; agg strip in SBUF (aggS bf16). Updates hT strip
            (pre-norm). ACT funcs used: Sigmoid/Copy/Tanh only."""
            s0, w = banks[b]
            rz = []
            for g in (0, 1):
                g_ps = ps.tile([128, BANK], f32, tag="ps", name=f"rz{l}{b}{g}")
                nc.tensor.matmul(
                    out=g_ps[:, :w], lhsT=wihT_sb[:, g * H: (g + 1) * H],
                    rhs=aggS[:, :w], start=True, stop=False,
                )
                nc.tensor.matmul(
                    out=g_ps[:, :w], lhsT=whhT_sb[:, g * H: (g + 1) * H],
                    rhs=hT16[:, s0: s0 + w], start=False, stop=True,
                )
                gt = gpool.tile([128, BANK], f32, tag=f"g{g}", name=f"gs{l}{b}{g}")
                nc.scalar.activation(
                    out=gt[:, :w], in_=g_ps[:, :w], func=AF.Sigmoid,
                    bias=brz_sb[:, g: g + 1],
                )
                rz.append(gt)
            r_t, z_t = rz
            in_ps = ps.tile([128, BANK], f32, tag="ps", name=f"in{l}{b}")
            nc.tensor.matmul(
                out=in_ps[:, :w], lhsT=binrow[:1, :], rhs=ones_row[:1, :w],
                start=True, stop=False,
            )
            nc.tensor.matmul(
                out=in_ps[:, :w], lhsT=wihT_sb[:, 2 * H: 3 * H],
                rhs=aggS[:, :w], start=False, stop=True,
            )
            i_n = gpool.tile([128, BANK], f32, tag="gin", name=f"gin{l}{b}")
            nc.scalar.activation(out=i_n[:, :w], in_=in_ps[:, :w], func=AF.Copy)
            hn_ps = ps.tile([128, BANK], f32, tag="ps", name=f"hn{l}{b}")
            nc.tensor.matmul(
                out=hn_ps[:, :w], lhsT=bhnrow[:1, :], rhs=ones_row[:1, :w],
                start=True, stop=False,
            )
            nc.tensor.matmul(
                out=hn_ps[:, :w], lhsT=whhT_sb[:, 2 * H: 3 * H],
                rhs=hT16[:, s0: s0 + w], start=False, stop=True,
            )
            h_n = gpool.tile([128, BANK], f32, tag="ghn", name=f"ghn{l}{b}")
            nc.scalar.activation(out=h_n[:, :w], in_=hn_ps[:, :w], func=AF.Copy)
            n_t = tpool.tile([128, BANK], f32, tag="n_t", name=f"n{l}{b}")
            nc.vector.tensor_tensor(
                out=n_t[:, :w], in0=r_t[:, :w], in1=h_n[:, :w], op=OP.mult
            )
            nc.vector.tensor_tensor(
                out=n_t[:, :w], in0=n_t[:, :w], in1=i_n[:, :w], op=OP.add
            )
            nc.scalar.activation(out=n_t[:, :w], in_=n_t[:, :w], func=AF.Tanh)
            d_t = tpool.tile([128, BANK], f32, tag="d_t", name=f"d{l}{b}")
            nc.vector.tensor_tensor(
                out=d_t[:, :w], in0=hT[:, s0: s0 + w], in1=n_t[:, :w],
                op=OP.subtract,
            )
            nc.vector.tensor_tensor(
                out=d_t[:, :w], in0=d_t[:, :w], in1=z_t[:, :w], op=OP.mult
            )
            nc.vector.tensor_tensor(
                out=hT[:, s0: s0 + w], in0=d_t[:, :w], in1=n_t[:, :w], op=OP.add
            )

        def store_chunk_and_allgather(l, c, h8):
            """Stage chunk c (pre-converted fp8 columns in h8) as INTERLEAVED
            pair rows (byte f*2+par) into ag_in[l] via one XBAR DMA
            transpose, then AllGather into tables[l] chunk-c row range."""
            p0, p1 = cb[c], cb[c + 1]
            csz = p1 - p0
            nblk = csz // 256
            wreal = min(npc, p1) - p0
            if wreal < csz:
                nc.vector.memset(h8[:, wreal:], 0.0)
            rows = xpool.tile(
                [128, nblk, 256], fp8, tag="xbuf", name=f"rows{l}_{c}"
            )
            nc.sync.dma_start_transpose(
                out=rows[:, :, :].bitcast(i16),
                in_=h8[:, :].bitcast(i16),
            )
            dst = (
                ag_in[l]
                .ap()[p0 // 2: p1 // 2, :, :]
                .rearrange("(t k) q f -> k t (q f)", k=128)
            )
            nc.sync.dma_start(out=dst, in_=rows[:, :, :])
            gr0 = cbases[c]
            gr1 = cbases[c] + CORES * csizes[c]
            nc.gpsimd.collective_compute(
                "AllGather",
                mybir.AluOpType.bypass,
                replica_groups=groups,
                ins=[ag_in[l].ap()[p0 // 2: p1 // 2, :, :]],
                outs=[tables[l].ap()[gr0 // 2: gr1 // 2, :]],
            )

        def decode_cols(q0, q1):
            """Decoder for position range [q0, q1) (tile-aligned)."""
            for t in range(q0 // 128, (q1 + 127) // 128):
                wt = min(128, npc - t * 128)
                if wt <= 0:
                    break
                o_ps = ps.tile([128, 128], f32, tag="ps", name=f"dec{t}")
                nc.tensor.matmul(
                    out=o_ps[:wt, :], lhsT=onesk1[:1, :wt], rhs=b2_sb[:1, :],
                    start=True, stop=False,
                )
                nc.tensor.matmul(
                    out=o_ps[:wt, :], lhsT=hmaxT[:, t * 128: t * 128 + wt],
                    rhs=w2T_sb[:], start=False, stop=True,
                )
                orow = tpool.tile([128, 128], f32, tag="orow", name=f"or{t}")
                nc.scalar.activation(
                    out=orow[:wt, :], in_=o_ps[:wt, :], func=AF.Copy
                )
                nc.sync.dma_start(
                    out=out_d[t * 128: t * 128 + wt, :], in_=orow[:wt, :]
                )

        # ---------------- encoder
        for b, (s0, w) in enumerate(banks):
            h0_ps = ps.tile([128, BANK], f32, tag="ps", name=f"enc{b}")
            nc.tensor.matmul(
                out=h0_ps[:, :w], lhsT=w1T_sb[:], rhs=aggF[0:3, s0: s0 + w],
                start=True, stop=True,
            )
            nc.scalar.activation(
                out=hT[:, s0: s0 + w], in_=h0_ps[:, :w], func=AF.Relu,
                bias=b1_sb[:, :1],
            )
            nc.vector.memset(hmaxT[:, s0: s0 + w], -1e30)
        for c in CH_ORDER:
            h8c = xpool.tile(
                [128, cb[c + 1] - cb[c]], fp8, tag="h8", name=f"h8e_{c}"
            )
            norm_phase(chunk_banks[c], h8=h8c, h8_base=cb[c])
            store_chunk_and_allgather(0, c, h8c)

        # ---------------- message-passing layers
        for l in range(L):
            first_phase = {}
            for p in CH_ORDER:
                for b in range(n_banks):
                    if (p, b) in pb_wins and b not in first_phase:
                        first_phase[b] = p

            def emit_gathers(p):
                """Emit the phase-p dma_gather calls (Pool queue order =
                gather stream order). Returns window -> (tile, slot)."""
                pw0, pw1 = phase_wins[p]
                pr0 = cbases[p] // 2
                pr1 = (cbases[p] + CORES * csizes[p]) // 2
                src_ap = tables[l].ap()[pr0:pr1, :]
                chunk_of = {}
                for w in range(pw0, pw1 + 1, CALL_WINDOWS):
                    wlast = min(w + CALL_WINDOWS - 1, pw1)
                    nidx = (wlast - w + 1) * WIN_EDGES
                    g_sb = epool.tile(
                        [128, CALL_WINDOWS * TPW, 256], fp8, tag="gbuf",
                        name=f"g{l}_{p}_{w}",
                    )
                    nc.gpsimd.dma_gather(
                        g_sb[:, : nidx // 128, :],
                        src_ap,
                        idx_sb[
                            :, w * WIN_EDGES // 16: (wlast + 1) * WIN_EDGES // 16
                        ],
                        nidx,
                        nidx,
                        256,
                    )
                    for ww in range(w, wlast + 1):
                        chunk_of[ww] = (g_sb, (ww - w) * TPW)
                return chunk_of

            def agg_bank(p, b, chunk_of):
                """Selector matmuls + eviction for (phase p, bank b).
                Returns the bf16 agg strip when p is the last phase."""
                s0, wb = banks[b]
                w_first, w_last = pb_wins[(p, b)]
                apb = ps.tile([128, BANK], f32, tag="ps", name=f"agg{l}_{p}_{b}")
                for w in range(w_first, w_last + 1):
                    _, _, a, e = windows[w]
                    S = e - a
                    g_sb, slot0 = chunk_of[w]
                    for j in range(TPW):
                        pp = j // (TPW // 2)
                        lt = g_sb[:, slot0 + j, :].rearrange(
                            "p (f two) -> p two f", two=2
                        )[:, pp, :]
                        nc.tensor.matmul(
                            out=apb[:, a - s0: a - s0 + S],
                            lhsT=lt,
                            rhs=sel_sb[
                                :, sel_off[w] + j * S: sel_off[w] + (j + 1) * S
                            ],
                            start=(j == 0),
                            stop=(j == TPW - 1),
                        )
                if p != CH_ORDER[-1]:
                    if first_phase[b] == p:
                        nc.scalar.activation(
                            out=aggF[:, s0: s0 + wb], in_=apb[:, :wb],
                            func=AF.Copy,
                        )
                    else:
                        nc.vector.tensor_tensor(
                            out=aggF[:, s0: s0 + wb], in0=aggF[:, s0: s0 + wb],
                            in1=apb[:, :wb], op=OP.add,
                        )
                    return None
                aggS = gpool.tile([128, BANK], bf16, tag="aggS", name=f"as{l}{b}")
                if first_phase[b] == CH_ORDER[-1]:
                    nc.scalar.activation(
                        out=aggS[:, :wb], in_=apb[:, :wb], func=AF.Copy
                    )
                else:
                    nc.vector.tensor_tensor(
                        out=aggS[:, :wb], in0=aggF[:, s0: s0 + wb],
                        in1=apb[:, :wb], op=OP.add,
                    )
                return aggS

            bank_order = [bb for c in CH_ORDER for bb in chunk_banks[c]]
            for p in CH_ORDER[:-1]:
                chunk_of = emit_gathers(p)
                for b in bank_order:
                    if (p, b) in pb_wins:
                        agg_bank(p, b, chunk_of)
            p_last = CH_ORDER[-1]
            chunk_of2 = emit_gathers(p_last)
            for c in CH_ORDER:
                for b in chunk_banks[c]:
                    aggS = agg_bank(p_last, b, chunk_of2)
                    gru_bank(l, b, aggS)
                if l < L - 1:
                    h8c = xpool.tile(
                        [128, cb[c + 1] - cb[c]], fp8, tag="h8",
                        name=f"h8_{l}_{c}",
                    )
                    norm_phase(chunk_banks[c], h8=h8c, h8_base=cb[c])
                    store_chunk_and_allgather(l + 1, c, h8c)
                else:
                    norm_phase(chunk_banks[c])
                    decode_cols(cb[c], cb[c + 1])

        stack.close()

    nc.compile()
    return nc


# ---------------------------------------------------------------- entry points
def _prep(inputs):
    import ml_dtypes

    x = np.asarray(inputs["x"], np.float32)
    edge_src = np.asarray(inputs["edge_src"], np.int64)
    edge_dst = np.asarray(inputs["edge_dst"], np.int64)
    n_nodes = x.shape[0]
    meta = _plan(edge_src, edge_dst, n_nodes)

    W1 = np.asarray(inputs["W1"], np.float32)
    b1 = np.asarray(inputs["b1"], np.float32)
    W_ih = np.asarray(inputs["W_ih"], np.float32)
    b_ih = np.asarray(inputs["b_ih"], np.float32)
    W_hh = np.asarray(inputs["W_hh"], np.float32)
    b_hh = np.asarray(inputs["b_hh"], np.float32)
    W2 = np.asarray(inputs["W2"], np.float32)
    b2 = np.asarray(inputs["b2"], np.float32)

    brz = np.stack(
        [
            b_ih[0:128] + b_hh[0:128],
            b_ih[128:256] + b_hh[128:256],
            b_ih[256:384],
            b_hh[256:384],
        ],
        axis=1,
    ).astype(np.float32)

    shared = dict(
        w1T=np.ascontiguousarray(W1.T),
        b1=np.ascontiguousarray(b1[:, None]),
        wihT=np.ascontiguousarray(W_ih.T).astype(ml_dtypes.bfloat16),
        whhT=np.ascontiguousarray(W_hh.T).astype(ml_dtypes.bfloat16),
        brz=brz,
        w2T=np.ascontiguousarray(W2.T),
        binrow=np.ascontiguousarray(b_ih[256:384][None, :]).astype(ml_dtypes.bfloat16),
        bhnrow=np.ascontiguousarray(b_hh[256:384][None, :]).astype(ml_dtypes.bfloat16),
        b2=np.ascontiguousarray(b2[None, :]),
    )
    in_maps = []
    for r in range(CORES):
        xr = x[meta["order_per_core"][r]]
        in_maps.append(
            dict(
                xT=np.ascontiguousarray(xr.T),
                idx=meta["idx_maps"][r],
                sel=meta["sel_maps"][r],
                **shared,
            )
        )
    return meta, in_maps


def _assemble(meta, results, n_nodes):
    npc = meta["npc"]
    out = np.empty((n_nodes, 128), np.float32)
    for r in range(CORES):
        out[meta["order_per_core"][r]] = results[r]["out"][:npc]
    return out


@functools.lru_cache(maxsize=1)
def _get_compiled(key):
    meta, in_maps = _PENDING[key]
    nc = _build(meta)
    return nc, meta, in_maps


_PENDING = {}


def kernel(**inputs):
    x = np.asarray(inputs["x"])
    n_nodes = x.shape[0]
    meta, in_maps = _prep(inputs)
    key = hash(
        (
            n_nodes,
            np.asarray(inputs["edge_src"]).tobytes(),
            np.asarray(inputs["edge_dst"]).tobytes(),
        )
    )
    _PENDING[key] = (meta, in_maps)
    nc, meta, _ = _get_compiled(key)

    from concourse.bass_utils import run_bass_kernel_spmd

    trace = bool(int(os.environ.get("KERNEL_TRACE", "0")))
    res = run_bass_kernel_spmd(
        nc, in_maps, core_ids=list(range(CORES)), trace=trace
    )
    kernel.last_results = res
    return _assemble(meta, res.results, n_nodes)


# revision 17
# speedup vs baseline: 1.2479x; 1.2479x over previous
"""DrBCNet GNN message-passing kernel for 8 Trainium2 NeuronCores — v5.

Strategy (dst-sharded, sparse gather + selector matmuls, 3-chunk pipeline):
  - Nodes globally degree-sorted, dealt round-robin to 8 cores (3750 each);
    GRU/norm/decoder node-local in column layout (hT [feat, pos]).
  - Node positions split into 3 chunks A|B|C (bank-aligned). The h row-table
    (fp8 pairs, [30720 rows, 128B]) lives in DRAM (Shared), rebuilt each
    layer by THREE chunk AllGathers (A, B, C) so collectives pipeline with
    the gather stream.
  - Edges are grouped by (src chunk = phase, dst bank): phase-p gathers read
    only the chunk-p row range of the table (chunk-relative indices), so
    they start as soon as AG_p lands while later AGs are still in flight.
    Per (phase, bank) partial aggregates accumulate via selector matmuls in
    PSUM, evicted into an SBUF f32 accumulator; after the last phase the
    bank's aggregate feeds the GRU. GRU/norm/AG(l+1) run chunk-by-chunk so
    AG_A(l+1) fires while chunk B/C of layer l still compute.
  - GRU: bf16 weight lhsT, i+h gate sums in PSUM, biases folded into the
    PSUM-evacuating activations; Sqrt (l2norm) phase-batched per chunk.
"""

import functools
import os

import numpy as np

CORES = 8
H = 128
L = 5
BANK = 512
NORM_EPS_SQ = 1e-24
WIN_EDGES = int(os.environ.get("WE", "512"))
TPW = WIN_EDGES // 128
HALF_W = WIN_EDGES // 2
# chunk A = leading A_COLS cols; B = [A_COLS, CHB*512); C = rest
A_COLS = int(os.environ.get("ACOLS", "256"))
CHB = int(os.environ.get("CHB", "5"))
CALL_WINDOWS = int(os.environ.get("CW", "2"))  # windows per dma_gather call
# processing order of chunks (chain + phase stream): small A first, big B last
CH_ORDER = [int(x) for x in os.environ.get("CHORD", "0,2,1").split(",")]


# ---------------------------------------------------------------- host planning
def _plan(edge_src, edge_dst, n_nodes):
    npc = n_nodes // CORES
    npc_pad = ((npc + 127) // 128) * 128
    ntiles = npc_pad // 128

    deg = np.bincount(edge_dst, minlength=n_nodes)
    gorder = np.argsort(-deg, kind="stable")
    gpos = np.empty(n_nodes, np.int64)
    gpos[gorder] = np.arange(n_nodes)
    owner = gpos % CORES
    pos = gpos // CORES
    order_per_core = [gorder[r::CORES] for r in range(CORES)]

    # chunks in per-core position space (128-aligned => even sizes)
    cb = [0, A_COLS, CHB * BANK, npc_pad]
    csizes = [cb[1] - cb[0], cb[2] - cb[1], cb[3] - cb[2]]
    # global table row base of each chunk (chunk-major layout)
    cbases = [0, CORES * csizes[0], CORES * (csizes[0] + csizes[1])]
    # bank col ranges: chunk A is one narrow bank, then pad to the B|C
    # boundary, then 512-wide banks
    bank_edges = [0, A_COLS]
    x = A_COLS
    while x < cb[2]:
        step = min(BANK, cb[2] - x)
        x += step
        bank_edges.append(x)
    while x < npc:
        step = min(BANK, npc - x)
        x += step
        bank_edges.append(x)
    banks = [
        (bank_edges[i], min(bank_edges[i + 1], npc) - bank_edges[i])
        for i in range(len(bank_edges) - 1)
    ]
    n_banks = len(banks)
    first_c_bank = next(i for i, (s0, _) in enumerate(banks) if s0 >= cb[2])
    chunk_banks = [
        [0],
        list(range(1, first_c_bank)),
        list(range(first_c_bank, n_banks)),
    ]

    cidx = np.where(pos < cb[1], 0, np.where(pos < cb[2], 1, 2))
    base = np.take(cbases, cidx)
    size = np.take(csizes, cidx)
    start = np.take(np.array(cb[:3]), cidx)
    tpos = base + owner * size + (pos - start)

    srcrow_all = tpos[edge_src]
    src_phase = cidx[edge_src]
    dpos = pos[edge_dst]
    down = owner[edge_dst]

    # per-core per-column counts split by (phase, parity of table row)
    tpar = srcrow_all % 2
    Cnt = np.zeros((3, 2, CORES, npc), np.int64)
    for r in range(CORES):
        m = down == r
        for p in range(3):
            for par in range(2):
                np.add.at(
                    Cnt[p, par, r], dpos[m & (src_phase == p) & (tpar == par)], 1
                )

    # windows per (phase, bank): greedy pack columns while per-core per-parity
    # count fits HALF_W. Stream order: phase in CH_ORDER, dst chunk in
    # CH_ORDER, bank ascending within chunk.
    windows = []  # list of (phase, bank, col_a, col_b)
    for p in CH_ORDER:
        for b in [bb for c in CH_ORDER for bb in chunk_banks[c]]:
            c0 = banks[b][0]
            c1 = c0 + banks[b][1]
            w0 = c0
            rune = np.zeros(CORES, np.int64)
            runo = np.zeros(CORES, np.int64)
            for q in range(c0, c1):
                ce = Cnt[p, 0, :, q]
                co = Cnt[p, 1, :, q]
                if (rune + ce).max() > HALF_W or (runo + co).max() > HALF_W:
                    windows.append((p, b, w0, q))
                    w0 = q
                    rune = ce.copy()
                    runo = co.copy()
                else:
                    rune += ce
                    runo += co
            windows.append((p, b, w0, c1))
    nwin = len(windows)
    win_width = [e - a for (_, _, a, e) in windows]
    sel_off = np.concatenate([[0], np.cumsum([TPW * w for w in win_width])])
    total_sel = int(sel_off[-1])
    total_idx = nwin * WIN_EDGES

    import ml_dtypes

    # per-core idx + selector maps. tiles 0..TPW/2-1 of each window: even
    # parity srcs; rest: odd. idx is the CHUNK-RELATIVE pair row
    # ((tpos - cbases[p]) // 2) so gathers can bind to the chunk row range.
    idx_maps, sel_maps = [], []
    for r in range(CORES):
        eidx = np.nonzero(down == r)[0]
        dp = dpos[eidx]
        sr = srcrow_all[eidx]
        ph = src_phase[eidx]
        key = ph * (npc * 2) + dp * 2 + (sr % 2)
        o = np.argsort(key, kind="stable")
        dp, sr, ph = dp[o], sr[o], ph[o]
        par = sr % 2
        skey = ph * (npc * 2) + dp * 2 + par  # sorted within each phase
        idxs = np.zeros(total_idx, np.int16)
        sel = np.zeros((128, total_sel), np.float32)
        for w, (p, b, a, e) in enumerate(windows):
            S = e - a
            base_i = w * WIN_EDGES
            lo = np.searchsorted(skey, p * (npc * 2) + a * 2)
            hi = np.searchsorted(skey, p * (npc * 2) + e * 2)
            seg = slice(lo, hi)
            pvals = par[seg]
            for pp in (0, 1):
                m = np.nonzero(pvals == pp)[0]
                cnt = len(m)
                assert cnt <= HALF_W, (r, w, pp, cnt)
                slot0 = base_i + pp * HALF_W
                rows_sr = sr[seg][m]
                idxs[slot0: slot0 + cnt] = (
                    (rows_sr - cbases[p]) // 2
                ).astype(np.int16)
                loc = (dp[seg][m] - a).astype(np.int64)
                j = np.arange(cnt) // 128 + (TPW // 2) * pp
                q = np.arange(cnt) % 128
                sel[q, sel_off[w] + j * S + loc] = 1.0
        idx_w = np.zeros((128, total_idx // 16), np.int16)
        wrapped = idxs.reshape(total_idx // 16, 16).T
        for g in range(8):
            idx_w[g * 16: (g + 1) * 16, :] = wrapped
        idx_maps.append(idx_w)
        sel_maps.append(sel.astype(ml_dtypes.float8_e4m3fn))

    # window index ranges per (phase, bank) and per phase
    pb_wins = {}
    phase_wins = {}
    for w, (p, b, a, e) in enumerate(windows):
        if (p, b) not in pb_wins:
            pb_wins[(p, b)] = [w, w]
        pb_wins[(p, b)][1] = w
        if p not in phase_wins:
            phase_wins[p] = [w, w]
        phase_wins[p][1] = w

    return dict(
        npc=npc,
        npc_pad=npc_pad,
        ntiles=ntiles,
        n_banks=n_banks,
        banks=banks,
        cb=cb,
        csizes=csizes,
        cbases=cbases,
        chunk_banks=chunk_banks,
        nwin=nwin,
        windows=windows,
        sel_off=sel_off,
        total_sel=total_sel,
        total_idx=total_idx,
        pb_wins=pb_wins,
        phase_wins=phase_wins,
        order_per_core=order_per_core,
        idx_maps=idx_maps,
        sel_maps=sel_maps,
    )


# ---------------------------------------------------------------- bass program
def _build(meta):
    import concourse.bacc as bacc
    import concourse.mybir as mybir
    import concourse.tile as tile
    from concourse import library_config

    npc = meta["npc"]
    npc_pad = meta["npc_pad"]
    ntiles = meta["ntiles"]
    n_banks = meta["n_banks"]
    banks = meta["banks"]
    cb = meta["cb"]
    csizes = meta["csizes"]
    cbases = meta["cbases"]
    chunk_banks = meta["chunk_banks"]
    windows = meta["windows"]
    sel_off = meta["sel_off"]
    total_sel = meta["total_sel"]
    total_idx = meta["total_idx"]
    pb_wins = meta["pb_wins"]
    phase_wins = meta["phase_wins"]
    n_tbl = CORES * npc_pad
    f32 = mybir.dt.float32
    bf16 = mybir.dt.bfloat16
    i16 = mybir.dt.int16
    fp8 = mybir.dt.float8e4
    AF = mybir.ActivationFunctionType
    OP = mybir.AluOpType

    nc = bacc.Bacc(
        "TRN2",
        target_bir_lowering=False,
        debug=False,
        num_devices=CORES,
        dynamic_dma_scratch_size=int(os.environ.get("SCR", "32768")),
    )

    # I/O
    xT_d = nc.dram_tensor("xT", [3, npc], f32, kind="ExternalInput")
    idx_d = nc.dram_tensor("idx", [128, total_idx // 16], i16, kind="ExternalInput")
    sel_d = nc.dram_tensor("sel", [128, total_sel], fp8, kind="ExternalInput")
    w1T_d = nc.dram_tensor("w1T", [3, 128], f32, kind="ExternalInput")
    b1_d = nc.dram_tensor("b1", [128, 1], f32, kind="ExternalInput")
    wihT_d = nc.dram_tensor("wihT", [128, 3 * H], bf16, kind="ExternalInput")
    whhT_d = nc.dram_tensor("whhT", [128, 3 * H], bf16, kind="ExternalInput")
    brz_d = nc.dram_tensor("brz", [128, 4], f32, kind="ExternalInput")  # br,bz,bin,bhn
    w2T_d = nc.dram_tensor("w2T", [128, 128], f32, kind="ExternalInput")
    b2_d = nc.dram_tensor("b2", [1, 128], f32, kind="ExternalInput")
    binrow_d = nc.dram_tensor("binrow", [1, 128], bf16, kind="ExternalInput")
    bhnrow_d = nc.dram_tensor("bhnrow", [1, 128], bf16, kind="ExternalInput")
    out_d = nc.dram_tensor("out", [npc_pad, 128], f32, kind="ExternalOutput")

    ag_in = [
        nc.dram_tensor(f"agin{l}", [npc_pad // 2, 2, 128], fp8) for l in range(L)
    ]
    tables = [
        nc.dram_tensor(f"table{l}", [n_tbl // 2, 256], fp8, addr_space="Shared")
        for l in range(L)
    ]
    groups = [list(range(CORES))]

    with tile.TileContext(nc) as tc:
        import contextlib

        stack = contextlib.ExitStack()
        nc.gpsimd.load_library(library_config.mlp)
        per = stack.enter_context(tc.tile_pool(name="per", bufs=1))

        def _T(shape, dtype, name=None):
            return per.tile(shape, dtype, name=name, tag=name)

        idx_sb = _T([128, total_idx // 16], i16, name="idx_sb")
        sel_sb = _T([128, total_sel], fp8, name="sel_sb")
        hT = _T([128, npc], f32, name="hT")
        hmaxT = _T([128, npc], f32, name="hmaxT")
        hT16 = _T([128, npc], bf16, name="hT16")
        aggF = _T([128, npc], f32, name="aggF")
        w1T_sb = _T([3, 128], f32, name="w1T_sb")
        b1_sb = _T([128, 1], f32, name="b1_sb")
        wihT_sb = _T([128, 3 * H], bf16, name="wihT_sb")
        whhT_sb = _T([128, 3 * H], bf16, name="whhT_sb")
        brz_sb = _T([128, 4], f32, name="brz_sb")
        w2T_sb = _T([128, 128], f32, name="w2T_sb")
        b2_sb = _T([1, 128], f32, name="b2_sb")
        ones_col = _T([128, 1], f32, name="ones_col")
        ones_row = _T([1, BANK], bf16, name="ones_row")
        binrow = _T([1, 128], bf16, name="binrow")
        bhnrow = _T([1, 128], bf16, name="bhnrow")
        onesk1 = _T([1, 128], f32, name="onesk1")
        eps_sb = _T([1, 1], f32, name="eps_sb")

        gpool = stack.enter_context(
            tc.tile_pool(name="gpool", bufs=int(os.environ.get("GB", "2")))
        )
        epool = stack.enter_context(
            tc.tile_pool(name="epool", bufs=int(os.environ.get("EB", "8")))
        )
        xpool = stack.enter_context(tc.tile_pool(name="xpool", bufs=2))
        tpool = stack.enter_context(
            tc.tile_pool(name="tpool", bufs=int(os.environ.get("TB", "2")))
        )
        ps = stack.enter_context(tc.tile_pool(name="ps", bufs=8, space="PSUM"))

        nc.sync.dma_start(out=aggF[0:3, :], in_=xT_d[:])
        nc.sync.dma_start(out=idx_sb[:], in_=idx_d[:])
        nc.sync.dma_start(out=w1T_sb[:], in_=w1T_d[:])
        nc.sync.dma_start(out=b1_sb[:], in_=b1_d[:])
        nc.sync.dma_start(out=wihT_sb[:], in_=wihT_d[:])
        nc.sync.dma_start(out=whhT_sb[:], in_=whhT_d[:])
        nc.sync.dma_start(out=brz_sb[:], in_=brz_d[:])
        nc.sync.dma_start(out=w2T_sb[:], in_=w2T_d[:])
        nc.sync.dma_start(out=b2_sb[:], in_=b2_d[:])
        nc.vector.memset(eps_sb[:], NORM_EPS_SQ)
        nc.vector.memset(ones_col[:], 1.0)
        nc.vector.memset(ones_row[:], 1.0)
        nc.sync.dma_start(out=binrow[:], in_=binrow_d[:])
        nc.sync.dma_start(out=bhnrow[:], in_=bhnrow_d[:])
        nc.vector.memset(onesk1[:], 1.0)
        nc.scalar.dma_start(out=sel_sb[:], in_=sel_d[:])

        def norm_phase(bank_list, h8=None, h8_base=0):
            """l2norm hT strips for several banks; single Sqrt table window.
            If h8 is given, also emit the fp8 column copy (staging input)
            right after each bank's hT update."""
            ns_list = []
            for b in bank_list:
                s0, w = banks[b]
                sq = tpool.tile([128, BANK], f32, tag="sq", name=f"sq{b}")
                nc.vector.tensor_tensor(
                    out=sq[:, :w], in0=hT[:, s0: s0 + w], in1=hT[:, s0: s0 + w],
                    op=OP.mult,
                )
                ns_ps = ps.tile([1, BANK], f32, tag="ps", name=f"ns{b}")
                nc.tensor.matmul(
                    out=ns_ps[:1, :w], lhsT=ones_col[:], rhs=sq[:, :w],
                    start=True, stop=True,
                )
                ns_list.append(ns_ps)
            inv_list = []
            for b, ns_ps in zip(bank_list, ns_list):
                s0, w = banks[b]
                srt = tpool.tile([1, BANK], f32, tag="srt", name=f"srt{b}")
                nc.scalar.activation(
                    out=srt[:1, :w], in_=ns_ps[:1, :w], func=AF.Sqrt,
                    bias=eps_sb[:1, :1],
                )
                inv_t = tpool.tile([1, BANK], f32, tag="inv_t", name=f"inv{b}")
                nc.vector.reciprocal(out=inv_t[:1, :w], in_=srt[:1, :w])
                inv_list.append(inv_t)
            for b, inv_t in zip(bank_list, inv_list):
                s0, w = banks[b]
                bc_ps = ps.tile([128, BANK], f32, tag="ps", name=f"bc{b}")
                nc.tensor.matmul(
                    out=bc_ps[:, :w], lhsT=onesk1[:1, :], rhs=inv_t[:1, :w],
                    start=True, stop=True,
                )
                nc.vector.tensor_tensor(
                    out=hT[:, s0: s0 + w], in0=hT[:, s0: s0 + w],
                    in1=bc_ps[:, :w], op=OP.mult,
                )
                nc.vector.tensor_copy(
                    out=hT16[:, s0: s0 + w], in_=hT[:, s0: s0 + w]
                )
                if h8 is not None:
                    nc.scalar.activation(
                        out=h8[:, s0 - h8_base: s0 - h8_base + w],
                        in_=hT[:, s0: s0 + w], func=AF.Copy,
                    )
                nc.vector.tensor_tensor(
                    out=hmaxT[:, s0: s0 + w], in0=hmaxT[:, s0: s0 + w],
                    in1=hT[:, s0: s0 + w], op=OP.max,
                )

        def gru_bank(l, b, aggS):
            """GRU for bank b; agg strip in SBUF (aggS bf16). Updates hT strip
            (pre-norm). ACT funcs used: Sigmoid/Copy/Tanh only."""
            s0, w = banks[b]
            rz = []
            for g in (0, 1):
                g_ps = ps.tile([128, BANK], f32, tag="ps", name=f"rz{l}{b}{g}")
                nc.tensor.matmul(
                    out=g_ps[:, :w], lhsT=wihT_sb[:, g * H: (g + 1) * H],
                    rhs=aggS[:, :w], start=True, stop=False,
                )
                nc.tensor.matmul(
                    out=g_ps[:, :w], lhsT=whhT_sb[:, g * H: (g + 1) * H],
                    rhs=hT16[:, s0: s0 + w], start=False, stop=True,
                )
                gt = gpool.tile([128, BANK], f32, tag=f"g{g}", name=f"gs{l}{b}{g}")
                nc.scalar.activation(
                    out=gt[:, :w], in_=g_ps[:, :w], func=AF.Sigmoid,
                    bias=brz_sb[:, g: g + 1],
                )
                rz.append(gt)
            r_t, z_t = rz
            in_ps = ps.tile([128, BANK], f32, tag="ps", name=f"in{l}{b}")
            nc.tensor.matmul(
                out=in_ps[:, :w], lhsT=binrow[:1, :], rhs=ones_row[:1, :w],
                start=True, stop=False,
            )
            nc.tensor.matmul(
                out=in_ps[:, :w], lhsT=wihT_sb[:, 2 * H: 3 * H],
                rhs=aggS[:, :w], start=False, stop=True,
            )
            i_n = gpool.tile([128, BANK], f32, tag="gin", name=f"gin{l}{b}")
            nc.scalar.activation(out=i_n[:, :w], in_=in_ps[:, :w], func=AF.Copy)
            hn_ps = ps.tile([128, BANK], f32, tag="ps", name=f"hn{l}{b}")
            nc.tensor.matmul(
                out=hn_ps[:, :w], lhsT=bhnrow[:1, :], rhs=ones_row[:1, :w],
                start=True, stop=False,
            )
            nc.tensor.matmul(
                out=hn_ps[:, :w], lhsT=whhT_sb[:, 2 * H: 3 * H],
                rhs=hT16[:, s0: s0 + w], start=False, stop=True,
            )
            h_n = gpool.tile([128, BANK], f32, tag="ghn", name=f"ghn{l}{b}")
            nc.scalar.activation(out=h_n[:, :w], in_=hn_ps[:, :w], func=AF.Copy)
            n_t = tpool.tile([128, BANK], f32, tag="n_t", name=f"n{l}{b}")
            nc.vector.tensor_tensor(
                out=n_t[:, :w], in0=r_t[:, :w], in1=h_n[:, :w], op=OP.mult
            )
            nc.vector.tensor_tensor(
                out=n_t[:, :w], in0=n_t[:, :w], in1=i_n[:, :w], op=OP.add
            )
            nc.scalar.activation(out=n_t[:, :w], in_=n_t[:, :w], func=AF.Tanh)
            d_t = tpool.tile([128, BANK], f32, tag="d_t", name=f"d{l}{b}")
            nc.vector.tensor_tensor(
                out=d_t[:, :w], in0=hT[:, s0: s0 + w], in1=n_t[:, :w],
                op=OP.subtract,
            )
            nc.vector.tensor_tensor(
                out=d_t[:, :w], in0=d_t[:, :w], in1=z_t[:, :w], op=OP.mult
            )
            nc.vector.tensor_tensor(
                out=hT[:, s0: s0 + w], in0=d_t[:, :w], in1=n_t[:, :w], op=OP.add
            )

        def store_chunk_and_allgather(l, c, h8):
            """Stage chunk c (pre-converted fp8 columns in h8) as INTERLEAVED
            pair rows (byte f*2+par) into ag_in[l] via one XBAR DMA
            transpose, then AllGather into tables[l] chunk-c row range."""
            p0, p1 = cb[c], cb[c + 1]
            csz = p1 - p0
            nblk = csz // 256
            wreal = min(npc, p1) - p0
            if wreal < csz:
                nc.vector.memset(h8[:, wreal:], 0.0)
            rows = xpool.tile(
                [128, nblk, 256], fp8, tag="xbuf", name=f"rows{l}_{c}"
            )
            nc.sync.dma_start_transpose(
                out=rows[:, :, :].bitcast(i16),
                in_=h8[:, :].bitcast(i16),
            )
            dst = (
                ag_in[l]
                .ap()[p0 // 2: p1 // 2, :, :]
                .rearrange("(t k) q f -> k t (q f)", k=128)
            )
            nc.sync.dma_start(out=dst, in_=rows[:, :, :])
            gr0 = cbases[c]
            gr1 = cbases[c] + CORES * csizes[c]
            nc.gpsimd.collective_compute(
                "AllGather",
                mybir.AluOpType.bypass,
                replica_groups=groups,
                ins=[ag_in[l].ap()[p0 // 2: p1 // 2, :, :]],
                outs=[tables[l].ap()[gr0 // 2: gr1 // 2, :]],
            )

        def decode_cols(q0, q1):
            """Decoder for position range [q0, q1) (tile-aligned)."""
            for t in range(q0 // 128, (q1 + 127) // 128):
                wt = min(128, npc - t * 128)
                if wt <= 0:
                    break
                o_ps = ps.tile([128, 128], f32, tag="ps", name=f"dec{t}")
                nc.tensor.matmul(
                    out=o_ps[:wt, :], lhsT=onesk1[:1, :wt], rhs=b2_sb[:1, :],
                    start=True, stop=False,
                )
                nc.tensor.matmul(
                    out=o_ps[:wt, :], lhsT=hmaxT[:, t * 128: t * 128 + wt],
                    rhs=w2T_sb[:], start=False, stop=True,
                )
                orow = tpool.tile([128, 128], f32, tag="orow", name=f"or{t}")
                nc.scalar.activation(
                    out=orow[:wt, :], in_=o_ps[:wt, :], func=AF.Copy
                )
                nc.sync.dma_start(
                    out=out_d[t * 128: t * 128 + wt, :], in_=orow[:wt, :]
                )

        # ---------------- encoder
        for b, (s0, w) in enumerate(banks):
            h0_ps = ps.tile([128, BANK], f32, tag="ps", name=f"enc{b}")
            nc.tensor.matmul(
                out=h0_ps[:, :w], lhsT=w1T_sb[:], rhs=aggF[0:3, s0: s0 + w],
                start=True, stop=True,
            )
            nc.scalar.activation(
                out=hT[:, s0: s0 + w], in_=h0_ps[:, :w], func=AF.Relu,
                bias=b1_sb[:, :1],
            )
            nc.vector.memset(hmaxT[:, s0: s0 + w], -1e30)
        for c in CH_ORDER:
            h8c = xpool.tile(
                [128, cb[c + 1] - cb[c]], fp8, tag="h8", name=f"h8e_{c}"
            )
            norm_phase(chunk_banks[c], h8=h8c, h8_base=cb[c])
            store_chunk_and_allgather(0, c, h8c)

        # ---------------- message-passing layers
        for l in range(L):
            first_phase = {}
            for p in CH_ORDER:
                for b in range(n_banks):
                    if (p, b) in pb_wins and b not in first_phase:
                        first_phase[b] = p

            def emit_gathers(p):
                """Emit the phase-p dma_gather calls (Pool queue order =
                gather stream order). Returns window -> (tile, slot)."""
                pw0, pw1 = phase_wins[p]
                pr0 = cbases[p] // 2
                pr1 = (cbases[p] + CORES * csizes[p]) // 2
                src_ap = tables[l].ap()[pr0:pr1, :]
                chunk_of = {}
                for w in range(pw0, pw1 + 1, CALL_WINDOWS):
                    wlast = min(w + CALL_WINDOWS - 1, pw1)
                    nidx = (wlast - w + 1) * WIN_EDGES
                    g_sb = epool.tile(
                        [128, CALL_WINDOWS * TPW, 256], fp8, tag="gbuf",
                        name=f"g{l}_{p}_{w}",
                    )
                    nc.gpsimd.dma_gather(
                        g_sb[:, : nidx // 128, :],
                        src_ap,
                        idx_sb[
                            :, w * WIN_EDGES // 16: (wlast + 1) * WIN_EDGES // 16
                        ],
                        nidx,
                        nidx,
                        256,
                    )
                    for ww in range(w, wlast + 1):
                        chunk_of[ww] = (g_sb, (ww - w) * TPW)
                return chunk_of

            def agg_bank(p, b, chunk_of):
                """Selector matmuls + eviction for (phase p, bank b).
                Returns the bf16 agg strip when p is the last phase."""
                s0, wb = banks[b]
                w_first, w_last = pb_wins[(p, b)]
                apb = ps.tile([128, BANK], f32, tag="ps", name=f"agg{l}_{p}_{b}")
                for w in range(w_first, w_last + 1):
                    _, _, a, e = windows[w]
                    S = e - a
                    g_sb, slot0 = chunk_of[w]
                    for j in range(TPW):
                        pp = j // (TPW // 2)
                        lt = g_sb[:, slot0 + j, :].rearrange(
                            "p (f two) -> p two f", two=2
                        )[:, pp, :]
                        nc.tensor.matmul(
                            out=apb[:, a - s0: a - s0 + S],
                            lhsT=lt,
                            rhs=sel_sb[
                                :, sel_off[w] + j * S: sel_off[w] + (j + 1) * S
                            ],
                            start=(j == 0),
                            stop=(j == TPW - 1),
                        )
                if p != CH_ORDER[-1]:
                    if first_phase[b] == p:
                        nc.scalar.activation(
                            out=aggF[:, s0: s0 + wb], in_=apb[:, :wb],
                            func=AF.Copy,
                        )
                    else:
                        nc.vector.tensor_tensor(
                            out=aggF[:, s0: s0 + wb], in0=aggF[:, s0: s0 + wb],
                            in1=apb[:, :wb], op=OP.add,
                        )
                    return None
                aggS = gpool.tile([128, BANK], bf16, tag="aggS", name=f"as{l}{b}")
                if first_phase[b] == CH_ORDER[-1]:
                    nc.scalar.activation(
                        out=aggS[:, :wb], in_=apb[:, :wb], func=AF.Copy
                    )
                else:
                    nc.vector.tensor_tensor(
                        out=aggS[:, :wb], in0=aggF[:, s0: s0 + wb],
                        in1=apb[:, :wb], op=OP.add,
                    )
                return aggS

            bank_order = [bb for c in CH_ORDER for bb in chunk_banks[c]]
            for p in CH_ORDER[:-1]:
                chunk_of = emit_gathers(p)
                for b in bank_order:
                    if (p, b) in pb_wins:
                        agg_bank(p, b, chunk_of)
            p_last = CH_ORDER[-1]
            chunk_of2 = emit_gathers(p_last)
            for c in CH_ORDER:
                for b in chunk_banks[c]:
                    aggS = agg_bank(p_last, b, chunk_of2)
                    gru_bank(l, b, aggS)
                if l < L - 1:
                    h8c = xpool.tile(
                        [128, cb[c + 1] - cb[c]], fp8, tag="h8",
                        name=f"h8_{l}_{c}",
                    )
                    norm_phase(chunk_banks[c], h8=h8c, h8_base=cb[c])
                    store_chunk_and_allgather(l + 1, c, h8c)
                else:
                    norm_phase(chunk_banks[c])
                    decode_cols(cb[c], cb[c + 1])

        stack.close()

    nc.compile()
    return nc


# ---------------------------------------------------------------- entry points
def _prep(inputs):
    import ml_dtypes

    x = np.asarray(inputs["x"], np.float32)
    edge_src = np.asarray(inputs["edge_src"], np.int64)
    edge_dst = np.asarray(inputs["edge_dst"], np.int64)
    n_nodes = x.shape[0]
    meta = _plan(edge_src, edge_dst, n_nodes)

    W1 = np.asarray(inputs["W1"], np.float32)
    b1 = np.asarray(inputs["b1"], np.float32)
    W_ih = np.asarray(inputs["W_ih"], np.float32)
    b_ih = np.asarray(inputs["b_ih"], np.float32)
    W_hh = np.asarray(inputs["W_hh"], np.float32)
    b_hh = np.asarray(inputs["b_hh"], np.float32)
    W2 = np.asarray(inputs["W2"], np.float32)
    b2 = np.asarray(inputs["b2"], np.float32)

    brz = np.stack(
        [
            b_ih[0:128] + b_hh[0:128],
            b_ih[128:256] + b_hh[128:256],
            b_ih[256:384],
            b_hh[256:384],
        ],
        axis=1,
    ).astype(np.float32)

    shared = dict(
        w1T=np.ascontiguousarray(W1.T),
        b1=np.ascontiguousarray(b1[:, None]),
        wihT=np.ascontiguousarray(W_ih.T).astype(ml_dtypes.bfloat16),
        whhT=np.ascontiguousarray(W_hh.T).astype(ml_dtypes.bfloat16),
        brz=brz,
        w2T=np.ascontiguousarray(W2.T),
        binrow=np.ascontiguousarray(b_ih[256:384][None, :]).astype(ml_dtypes.bfloat16),
        bhnrow=np.ascontiguousarray(b_hh[256:384][None, :]).astype(ml_dtypes.bfloat16),
        b2=np.ascontiguousarray(b2[None, :]),
    )
    in_maps = []
    for r in range(CORES):
        xr = x[meta["order_per_core"][r]]
        in_maps.append(
            dict(
                xT=np.ascontiguousarray(xr.T),
                idx=meta["idx_maps"][r],
                sel=meta["sel_maps"][r],
                **shared,
            )
        )
    return meta, in_maps


def _assemble(meta, results, n_nodes):
    npc = meta["npc"]
    out = np.empty((n_nodes, 128), np.float32)
    for r in range(CORES):
        out[meta["order_per_core"][r]] = results[r]["out"][:npc]
    return out


@functools.lru_cache(maxsize=1)
def _get_compiled(key):
    meta, in_maps = _PENDING[key]
    nc = _build(meta)
    return nc, meta, in_maps


_PENDING = {}


def kernel(**inputs):
    x = np.asarray(inputs["x"])
    n_nodes = x.shape[0]
    meta, in_maps = _prep(inputs)
    key = hash(
        (
            n_nodes,
            np.asarray(inputs["edge_src"]).tobytes(),
            np.asarray(inputs["edge_dst"]).tobytes(),
        )
    )
    _PENDING[key] = (meta, in_maps)
    nc, meta, _ = _get_compiled(key)

    from concourse.bass_utils import run_bass_kernel_spmd

    trace = bool(int(os.environ.get("KERNEL_TRACE", "0")))
    res = run_bass_kernel_spmd(
        nc, in_maps, core_ids=list(range(CORES)), trace=trace
    )
    kernel.last_results = res
    return _assemble(meta, res.results, n_nodes)


# revision 18
# speedup vs baseline: 1.2613x; 1.0108x over previous
"""DrBCNet GNN message-passing kernel for 8 Trainium2 NeuronCores — v5.

Strategy (dst-sharded, sparse gather + selector matmuls, 3-chunk pipeline):
  - Nodes globally degree-sorted, dealt round-robin to 8 cores (3750 each);
    GRU/norm/decoder node-local in column layout (hT [feat, pos]).
  - Node positions split into 3 chunks A|B|C (bank-aligned). The h row-table
    (fp8 pairs, [30720 rows, 128B]) lives in DRAM (Shared), rebuilt each
    layer by THREE chunk AllGathers (A, B, C) so collectives pipeline with
    the gather stream.
  - Edges are grouped by (src chunk = phase, dst bank): phase-p gathers read
    only the chunk-p row range of the table (chunk-relative indices), so
    they start as soon as AG_p lands while later AGs are still in flight.
    Per (phase, bank) partial aggregates accumulate via selector matmuls in
    PSUM, evicted into an SBUF f32 accumulator; after the last phase the
    bank's aggregate feeds the GRU. GRU/norm/AG(l+1) run chunk-by-chunk so
    AG_A(l+1) fires while chunk B/C of layer l still compute.
  - GRU: bf16 weight lhsT, i+h gate sums in PSUM, biases folded into the
    PSUM-evacuating activations; Sqrt (l2norm) phase-batched per chunk.
"""

import functools
import os

import numpy as np

CORES = 8
H = 128
L = 5
BANK = 512
NORM_EPS_SQ = 1e-24
WIN_EDGES = int(os.environ.get("WE", "512"))
TPW = WIN_EDGES // 128
HALF_W = WIN_EDGES // 2
# chunk A = leading A_COLS cols; B = [A_COLS, CHB*512); C = rest
A_COLS = int(os.environ.get("ACOLS", "256"))
CHB = int(os.environ.get("CHB", "5"))
CALL_WINDOWS = int(os.environ.get("CW", "2"))  # windows per dma_gather call
# processing order of chunks (chain + phase stream): small A first, big B last
CH_ORDER = [int(x) for x in os.environ.get("CHORD", "0,2,1").split(",")]


# ---------------------------------------------------------------- host planning
def _plan(edge_src, edge_dst, n_nodes):
    npc = n_nodes // CORES
    npc_pad = ((npc + 127) // 128) * 128
    ntiles = npc_pad // 128

    deg = np.bincount(edge_dst, minlength=n_nodes)
    gorder = np.argsort(-deg, kind="stable")
    gpos = np.empty(n_nodes, np.int64)
    gpos[gorder] = np.arange(n_nodes)
    owner = gpos % CORES
    pos = gpos // CORES
    order_per_core = [gorder[r::CORES] for r in range(CORES)]

    # chunks in per-core position space (128-aligned => even sizes)
    cb = [0, A_COLS, CHB * BANK, npc_pad]
    csizes = [cb[1] - cb[0], cb[2] - cb[1], cb[3] - cb[2]]
    # global table row base of each chunk (chunk-major layout)
    cbases = [0, CORES * csizes[0], CORES * (csizes[0] + csizes[1])]
    # bank col ranges: chunk A is one narrow bank, then pad to the B|C
    # boundary, then 512-wide banks
    bank_edges = [0, A_COLS]
    x = A_COLS
    while x < cb[2]:
        step = min(BANK, cb[2] - x)
        x += step
        bank_edges.append(x)
    while x < npc:
        step = min(BANK, npc - x)
        x += step
        bank_edges.append(x)
    banks = [
        (bank_edges[i], min(bank_edges[i + 1], npc) - bank_edges[i])
        for i in range(len(bank_edges) - 1)
    ]
    n_banks = len(banks)
    first_c_bank = next(i for i, (s0, _) in enumerate(banks) if s0 >= cb[2])
    chunk_banks = [
        [0],
        list(range(1, first_c_bank)),
        list(range(first_c_bank, n_banks)),
    ]

    cidx = np.where(pos < cb[1], 0, np.where(pos < cb[2], 1, 2))
    base = np.take(cbases, cidx)
    size = np.take(csizes, cidx)
    start = np.take(np.array(cb[:3]), cidx)
    tpos = base + owner * size + (pos - start)

    srcrow_all = tpos[edge_src]
    src_phase = cidx[edge_src]
    dpos = pos[edge_dst]
    down = owner[edge_dst]

    # per-core per-column counts split by (phase, parity of table row)
    tpar = srcrow_all % 2
    Cnt = np.zeros((3, 2, CORES, npc), np.int64)
    for r in range(CORES):
        m = down == r
        for p in range(3):
            for par in range(2):
                np.add.at(
                    Cnt[p, par, r], dpos[m & (src_phase == p) & (tpar == par)], 1
                )

    # windows per (phase, dst chunk): greedy pack columns across the whole
    # chunk (bank boundaries handled by splitting matmuls in _build).
    # Stream order: phase in CH_ORDER, dst chunk in CH_ORDER.
    windows = []  # list of (phase, chunk, col_a, col_b)
    for p in CH_ORDER:
        for c in CH_ORDER:
            c0 = cb[c]
            c1 = min(cb[c + 1], npc)
            w0 = c0
            rune = np.zeros(CORES, np.int64)
            runo = np.zeros(CORES, np.int64)
            for q in range(c0, c1):
                ce = Cnt[p, 0, :, q]
                co = Cnt[p, 1, :, q]
                if (rune + ce).max() > HALF_W or (runo + co).max() > HALF_W:
                    windows.append((p, c, w0, q))
                    w0 = q
                    rune = ce.copy()
                    runo = co.copy()
                else:
                    rune += ce
                    runo += co
            windows.append((p, c, w0, c1))
    nwin = len(windows)
    win_width = [e - a for (_, _, a, e) in windows]
    sel_off = np.concatenate([[0], np.cumsum([TPW * w for w in win_width])])
    total_sel = int(sel_off[-1])
    total_idx = nwin * WIN_EDGES

    import ml_dtypes

    # per-core idx + selector maps. tiles 0..TPW/2-1 of each window: even
    # parity srcs; rest: odd. idx is the CHUNK-RELATIVE pair row
    # ((tpos - cbases[p]) // 2) so gathers can bind to the chunk row range.
    idx_maps, sel_maps = [], []
    for r in range(CORES):
        eidx = np.nonzero(down == r)[0]
        dp = dpos[eidx]
        sr = srcrow_all[eidx]
        ph = src_phase[eidx]
        key = ph * (npc * 2) + dp * 2 + (sr % 2)
        o = np.argsort(key, kind="stable")
        dp, sr, ph = dp[o], sr[o], ph[o]
        par = sr % 2
        skey = ph * (npc * 2) + dp * 2 + par  # sorted within each phase
        idxs = np.zeros(total_idx, np.int16)
        sel = np.zeros((128, total_sel), np.float32)
        for w, (p, b, a, e) in enumerate(windows):
            S = e - a
            base_i = w * WIN_EDGES
            lo = np.searchsorted(skey, p * (npc * 2) + a * 2)
            hi = np.searchsorted(skey, p * (npc * 2) + e * 2)
            seg = slice(lo, hi)
            pvals = par[seg]
            for pp in (0, 1):
                m = np.nonzero(pvals == pp)[0]
                cnt = len(m)
                assert cnt <= HALF_W, (r, w, pp, cnt)
                slot0 = base_i + pp * HALF_W
                rows_sr = sr[seg][m]
                idxs[slot0: slot0 + cnt] = (
                    (rows_sr - cbases[p]) // 2
                ).astype(np.int16)
                loc = (dp[seg][m] - a).astype(np.int64)
                j = np.arange(cnt) // 128 + (TPW // 2) * pp
                q = np.arange(cnt) % 128
                sel[q, sel_off[w] + j * S + loc] = 1.0
        idx_w = np.zeros((128, total_idx // 16), np.int16)
        wrapped = idxs.reshape(total_idx // 16, 16).T
        for g in range(8):
            idx_w[g * 16: (g + 1) * 16, :] = wrapped
        idx_maps.append(idx_w)
        sel_maps.append(sel.astype(ml_dtypes.float8_e4m3fn))

    # per (phase, bank): windows overlapping the bank with clipped col range
    pb_wins = {}
    phase_wins = {}
    for w, (p, c, a, e) in enumerate(windows):
        for b, (s0, wd) in enumerate(banks):
            lo = max(a, s0)
            hi = min(e, s0 + wd)
            if lo < hi:
                pb_wins.setdefault((p, b), []).append((w, lo, hi))
        if p not in phase_wins:
            phase_wins[p] = [w, w]
        phase_wins[p][1] = w

    return dict(
        npc=npc,
        npc_pad=npc_pad,
        ntiles=ntiles,
        n_banks=n_banks,
        banks=banks,
        cb=cb,
        csizes=csizes,
        cbases=cbases,
        chunk_banks=chunk_banks,
        nwin=nwin,
        windows=windows,
        sel_off=sel_off,
        total_sel=total_sel,
        total_idx=total_idx,
        pb_wins=pb_wins,
        phase_wins=phase_wins,
        order_per_core=order_per_core,
        idx_maps=idx_maps,
        sel_maps=sel_maps,
    )


# ---------------------------------------------------------------- bass program
def _build(meta):
    import concourse.bacc as bacc
    import concourse.mybir as mybir
    import concourse.tile as tile
    from concourse import library_config

    npc = meta["npc"]
    npc_pad = meta["npc_pad"]
    ntiles = meta["ntiles"]
    n_banks = meta["n_banks"]
    banks = meta["banks"]
    cb = meta["cb"]
    csizes = meta["csizes"]
    cbases = meta["cbases"]
    chunk_banks = meta["chunk_banks"]
    windows = meta["windows"]
    sel_off = meta["sel_off"]
    total_sel = meta["total_sel"]
    total_idx = meta["total_idx"]
    pb_wins = meta["pb_wins"]
    phase_wins = meta["phase_wins"]
    n_tbl = CORES * npc_pad
    f32 = mybir.dt.float32
    bf16 = mybir.dt.bfloat16
    i16 = mybir.dt.int16
    fp8 = mybir.dt.float8e4
    AF = mybir.ActivationFunctionType
    OP = mybir.AluOpType

    nc = bacc.Bacc(
        "TRN2",
        target_bir_lowering=False,
        debug=False,
        num_devices=CORES,
        dynamic_dma_scratch_size=int(os.environ.get("SCR", "32768")),
    )

    # I/O
    xT_d = nc.dram_tensor("xT", [3, npc], f32, kind="ExternalInput")
    idx_d = nc.dram_tensor("idx", [128, total_idx // 16], i16, kind="ExternalInput")
    sel_d = nc.dram_tensor("sel", [128, total_sel], fp8, kind="ExternalInput")
    w1T_d = nc.dram_tensor("w1T", [3, 128], f32, kind="ExternalInput")
    b1_d = nc.dram_tensor("b1", [128, 1], f32, kind="ExternalInput")
    wihT_d = nc.dram_tensor("wihT", [128, 3 * H], bf16, kind="ExternalInput")
    whhT_d = nc.dram_tensor("whhT", [128, 3 * H], bf16, kind="ExternalInput")
    brz_d = nc.dram_tensor("brz", [128, 4], f32, kind="ExternalInput")  # br,bz,bin,bhn
    w2T_d = nc.dram_tensor("w2T", [128, 128], f32, kind="ExternalInput")
    b2_d = nc.dram_tensor("b2", [1, 128], f32, kind="ExternalInput")
    binrow_d = nc.dram_tensor("binrow", [1, 128], bf16, kind="ExternalInput")
    bhnrow_d = nc.dram_tensor("bhnrow", [1, 128], bf16, kind="ExternalInput")
    out_d = nc.dram_tensor("out", [npc_pad, 128], f32, kind="ExternalOutput")

    ag_in = [
        nc.dram_tensor(f"agin{l}", [npc_pad // 2, 2, 128], fp8) for l in range(L)
    ]
    tables = [
        nc.dram_tensor(f"table{l}", [n_tbl // 2, 256], fp8, addr_space="Shared")
        for l in range(L)
    ]
    groups = [list(range(CORES))]

    with tile.TileContext(nc) as tc:
        import contextlib

        stack = contextlib.ExitStack()
        nc.gpsimd.load_library(library_config.mlp)
        per = stack.enter_context(tc.tile_pool(name="per", bufs=1))

        def _T(shape, dtype, name=None):
            return per.tile(shape, dtype, name=name, tag=name)

        idx_sb = _T([128, total_idx // 16], i16, name="idx_sb")
        sel_sb = _T([128, total_sel], fp8, name="sel_sb")
        hT = _T([128, npc], f32, name="hT")
        hmaxT = _T([128, npc], f32, name="hmaxT")
        hT16 = _T([128, npc], bf16, name="hT16")
        aggF = _T([128, npc], f32, name="aggF")
        w1T_sb = _T([3, 128], f32, name="w1T_sb")
        b1_sb = _T([128, 1], f32, name="b1_sb")
        wihT_sb = _T([128, 3 * H], bf16, name="wihT_sb")
        whhT_sb = _T([128, 3 * H], bf16, name="whhT_sb")
        brz_sb = _T([128, 4], f32, name="brz_sb")
        w2T_sb = _T([128, 128], f32, name="w2T_sb")
        b2_sb = _T([1, 128], f32, name="b2_sb")
        ones_col = _T([128, 1], f32, name="ones_col")
        ones_row = _T([1, BANK], bf16, name="ones_row")
        binrow = _T([1, 128], bf16, name="binrow")
        bhnrow = _T([1, 128], bf16, name="bhnrow")
        onesk1 = _T([1, 128], f32, name="onesk1")
        eps_sb = _T([1, 1], f32, name="eps_sb")

        gpool = stack.enter_context(
            tc.tile_pool(name="gpool", bufs=int(os.environ.get("GB", "2")))
        )
        epool = stack.enter_context(
            tc.tile_pool(name="epool", bufs=int(os.environ.get("EB", "8")))
        )
        xpool = stack.enter_context(tc.tile_pool(name="xpool", bufs=2))
        tpool = stack.enter_context(
            tc.tile_pool(name="tpool", bufs=int(os.environ.get("TB", "2")))
        )
        ps = stack.enter_context(tc.tile_pool(name="ps", bufs=8, space="PSUM"))

        nc.sync.dma_start(out=aggF[0:3, :], in_=xT_d[:])
        nc.sync.dma_start(out=idx_sb[:], in_=idx_d[:])
        nc.sync.dma_start(out=w1T_sb[:], in_=w1T_d[:])
        nc.sync.dma_start(out=b1_sb[:], in_=b1_d[:])
        nc.sync.dma_start(out=wihT_sb[:], in_=wihT_d[:])
        nc.sync.dma_start(out=whhT_sb[:], in_=whhT_d[:])
        nc.sync.dma_start(out=brz_sb[:], in_=brz_d[:])
        nc.sync.dma_start(out=w2T_sb[:], in_=w2T_d[:])
        nc.sync.dma_start(out=b2_sb[:], in_=b2_d[:])
        nc.vector.memset(eps_sb[:], NORM_EPS_SQ)
        nc.vector.memset(ones_col[:], 1.0)
        nc.vector.memset(ones_row[:], 1.0)
        nc.sync.dma_start(out=binrow[:], in_=binrow_d[:])
        nc.sync.dma_start(out=bhnrow[:], in_=bhnrow_d[:])
        nc.vector.memset(onesk1[:], 1.0)
        nc.scalar.dma_start(out=sel_sb[:], in_=sel_d[:])

        def norm_phase(bank_list, h8=None, h8_base=0):
            """l2norm hT strips for several banks; single Sqrt table window.
            If h8 is given, also emit the fp8 column copy (staging input)
            right after each bank's hT update."""
            ns_list = []
            for b in bank_list:
                s0, w = banks[b]
                sq = tpool.tile([128, BANK], f32, tag="sq", name=f"sq{b}")
                nc.vector.tensor_tensor(
                    out=sq[:, :w], in0=hT[:, s0: s0 + w], in1=hT[:, s0: s0 + w],
                    op=OP.mult,
                )
                ns_ps = ps.tile([1, BANK], f32, tag="ps", name=f"ns{b}")
                nc.tensor.matmul(
                    out=ns_ps[:1, :w], lhsT=ones_col[:], rhs=sq[:, :w],
                    start=True, stop=True,
                )
                ns_list.append(ns_ps)
            inv_list = []
            for b, ns_ps in zip(bank_list, ns_list):
                s0, w = banks[b]
                srt = tpool.tile([1, BANK], f32, tag="srt", name=f"srt{b}")
                nc.scalar.activation(
                    out=srt[:1, :w], in_=ns_ps[:1, :w], func=AF.Sqrt,
                    bias=eps_sb[:1, :1],
                )
                inv_t = tpool.tile([1, BANK], f32, tag="inv_t", name=f"inv{b}")
                nc.vector.reciprocal(out=inv_t[:1, :w], in_=srt[:1, :w])
                inv_list.append(inv_t)
            for b, inv_t in zip(bank_list, inv_list):
                s0, w = banks[b]
                bc_ps = ps.tile([128, BANK], f32, tag="ps", name=f"bc{b}")
                nc.tensor.matmul(
                    out=bc_ps[:, :w], lhsT=onesk1[:1, :], rhs=inv_t[:1, :w],
                    start=True, stop=True,
                )
                nc.vector.tensor_tensor(
                    out=hT[:, s0: s0 + w], in0=hT[:, s0: s0 + w],
                    in1=bc_ps[:, :w], op=OP.mult,
                )
                nc.vector.tensor_copy(
                    out=hT16[:, s0: s0 + w], in_=hT[:, s0: s0 + w]
                )
                if h8 is not None:
                    nc.scalar.activation(
                        out=h8[:, s0 - h8_base: s0 - h8_base + w],
                        in_=hT[:, s0: s0 + w], func=AF.Copy,
                    )
                nc.vector.tensor_tensor(
                    out=hmaxT[:, s0: s0 + w], in0=hmaxT[:, s0: s0 + w],
                    in1=hT[:, s0: s0 + w], op=OP.max,
                )

        def gru_bank(l, b, aggS):
            """GRU for bank b; agg strip in SBUF (aggS bf16). Updates hT strip
            (pre-norm). ACT funcs used: Sigmoid/Copy/Tanh only."""
            s0, w = banks[b]
            rz = []
            for g in (0, 1):
                g_ps = ps.tile([128, BANK], f32, tag="ps", name=f"rz{l}{b}{g}")
                nc.tensor.matmul(
                    out=g_ps[:, :w], lhsT=wihT_sb[:, g * H: (g + 1) * H],
                    rhs=aggS[:, :w], start=True, stop=False,
                )
                nc.tensor.matmul(
                    out=g_ps[:, :w], lhsT=whhT_sb[:, g * H: (g + 1) * H],
                    rhs=hT16[:, s0: s0 + w], start=False, stop=True,
                )
                gt = gpool.tile([128, BANK], f32, tag=f"g{g}", name=f"gs{l}{b}{g}")
                nc.scalar.activation(
                    out=gt[:, :w], in_=g_ps[:, :w], func=AF.Sigmoid,
                    bias=brz_sb[:, g: g + 1],
                )
                rz.append(gt)
            r_t, z_t = rz
            in_ps = ps.tile([128, BANK], f32, tag="ps", name=f"in{l}{b}")
            nc.tensor.matmul(
                out=in_ps[:, :w], lhsT=binrow[:1, :], rhs=ones_row[:1, :w],
                start=True, stop=False,
            )
            nc.tensor.matmul(
                out=in_ps[:, :w], lhsT=wihT_sb[:, 2 * H: 3 * H],
                rhs=aggS[:, :w], start=False, stop=True,
            )
            i_n = gpool.tile([128, BANK], f32, tag="gin", name=f"gin{l}{b}")
            nc.scalar.activation(out=i_n[:, :w], in_=in_ps[:, :w], func=AF.Copy)
            hn_ps = ps.tile([128, BANK], f32, tag="ps", name=f"hn{l}{b}")
            nc.tensor.matmul(
                out=hn_ps[:, :w], lhsT=bhnrow[:1, :], rhs=ones_row[:1, :w],
                start=True, stop=False,
            )
            nc.tensor.matmul(
                out=hn_ps[:, :w], lhsT=whhT_sb[:, 2 * H: 3 * H],
                rhs=hT16[:, s0: s0 + w], start=False, stop=True,
            )
            h_n = gpool.tile([128, BANK], f32, tag="ghn", name=f"ghn{l}{b}")
            nc.scalar.activation(out=h_n[:, :w], in_=hn_ps[:, :w], func=AF.Copy)
            n_t = tpool.tile([128, BANK], f32, tag="n_t", name=f"n{l}{b}")
            nc.vector.tensor_tensor(
                out=n_t[:, :w], in0=r_t[:, :w], in1=h_n[:, :w], op=OP.mult
            )
            nc.vector.tensor_tensor(
                out=n_t[:, :w], in0=n_t[:, :w], in1=i_n[:, :w], op=OP.add
            )
            nc.scalar.activation(out=n_t[:, :w], in_=n_t[:, :w], func=AF.Tanh)
            d_t = tpool.tile([128, BANK], f32, tag="d_t", name=f"d{l}{b}")
            nc.vector.tensor_tensor(
                out=d_t[:, :w], in0=hT[:, s0: s0 + w], in1=n_t[:, :w],
                op=OP.subtract,
            )
            nc.vector.tensor_tensor(
                out=d_t[:, :w], in0=d_t[:, :w], in1=z_t[:, :w], op=OP.mult
            )
            nc.vector.tensor_tensor(
                out=hT[:, s0: s0 + w], in0=d_t[:, :w], in1=n_t[:, :w], op=OP.add
            )

        def store_chunk_and_allgather(l, c, h8):
            """Stage chunk c (pre-converted fp8 columns in h8) as INTERLEAVED
            pair rows (byte f*2+par) into ag_in[l] via one XBAR DMA
            transpose, then AllGather into tables[l] chunk-c row range."""
            p0, p1 = cb[c], cb[c + 1]
            csz = p1 - p0
            nblk = csz // 256
            wreal = min(npc, p1) - p0
            if wreal < csz:
                nc.vector.memset(h8[:, wreal:], 0.0)
            rows = xpool.tile(
                [128, nblk, 256], fp8, tag="xbuf", name=f"rows{l}_{c}"
            )
            nc.sync.dma_start_transpose(
                out=rows[:, :, :].bitcast(i16),
                in_=h8[:, :].bitcast(i16),
            )
            dst = (
                ag_in[l]
                .ap()[p0 // 2: p1 // 2, :, :]
                .rearrange("(t k) q f -> k t (q f)", k=128)
            )
            nc.sync.dma_start(out=dst, in_=rows[:, :, :])
            gr0 = cbases[c]
            gr1 = cbases[c] + CORES * csizes[c]
            nc.gpsimd.collective_compute(
                "AllGather",
                mybir.AluOpType.bypass,
                replica_groups=groups,
                ins=[ag_in[l].ap()[p0 // 2: p1 // 2, :, :]],
                outs=[tables[l].ap()[gr0 // 2: gr1 // 2, :]],
            )

        def decode_cols(q0, q1):
            """Decoder for position range [q0, q1) (tile-aligned)."""
            for t in range(q0 // 128, (q1 + 127) // 128):
                wt = min(128, npc - t * 128)
                if wt <= 0:
                    break
                o_ps = ps.tile([128, 128], f32, tag="ps", name=f"dec{t}")
                nc.tensor.matmul(
                    out=o_ps[:wt, :], lhsT=onesk1[:1, :wt], rhs=b2_sb[:1, :],
                    start=True, stop=False,
                )
                nc.tensor.matmul(
                    out=o_ps[:wt, :], lhsT=hmaxT[:, t * 128: t * 128 + wt],
                    rhs=w2T_sb[:], start=False, stop=True,
                )
                orow = tpool.tile([128, 128], f32, tag="orow", name=f"or{t}")
                nc.scalar.activation(
                    out=orow[:wt, :], in_=o_ps[:wt, :], func=AF.Copy
                )
                nc.sync.dma_start(
                    out=out_d[t * 128: t * 128 + wt, :], in_=orow[:wt, :]
                )

        # ---------------- encoder
        for b, (s0, w) in enumerate(banks):
            h0_ps = ps.tile([128, BANK], f32, tag="ps", name=f"enc{b}")
            nc.tensor.matmul(
                out=h0_ps[:, :w], lhsT=w1T_sb[:], rhs=aggF[0:3, s0: s0 + w],
                start=True, stop=True,
            )
            nc.scalar.activation(
                out=hT[:, s0: s0 + w], in_=h0_ps[:, :w], func=AF.Relu,
                bias=b1_sb[:, :1],
            )
            nc.vector.memset(hmaxT[:, s0: s0 + w], -1e30)
        for c in CH_ORDER:
            h8c = xpool.tile(
                [128, cb[c + 1] - cb[c]], fp8, tag="h8", name=f"h8e_{c}"
            )
            norm_phase(chunk_banks[c], h8=h8c, h8_base=cb[c])
            store_chunk_and_allgather(0, c, h8c)

        # ---------------- message-passing layers
        for l in range(L):
            first_phase = {}
            for p in CH_ORDER:
                for b in range(n_banks):
                    if (p, b) in pb_wins and b not in first_phase:
                        first_phase[b] = p

            def emit_gathers(p):
                """Emit the phase-p dma_gather calls (Pool queue order =
                gather stream order). Returns window -> (tile, slot)."""
                pw0, pw1 = phase_wins[p]
                pr0 = cbases[p] // 2
                pr1 = (cbases[p] + CORES * csizes[p]) // 2
                src_ap = tables[l].ap()[pr0:pr1, :]
                chunk_of = {}
                for w in range(pw0, pw1 + 1, CALL_WINDOWS):
                    wlast = min(w + CALL_WINDOWS - 1, pw1)
                    nidx = (wlast - w + 1) * WIN_EDGES
                    g_sb = epool.tile(
                        [128, CALL_WINDOWS * TPW, 256], fp8, tag="gbuf",
                        name=f"g{l}_{p}_{w}",
                    )
                    nc.gpsimd.dma_gather(
                        g_sb[:, : nidx // 128, :],
                        src_ap,
                        idx_sb[
                            :, w * WIN_EDGES // 16: (wlast + 1) * WIN_EDGES // 16
                        ],
                        nidx,
                        nidx,
                        256,
                    )
                    for ww in range(w, wlast + 1):
                        chunk_of[ww] = (g_sb, (ww - w) * TPW)
                return chunk_of

            def agg_bank(p, b, chunk_of):
                """Selector matmuls + eviction for (phase p, bank b).
                Returns the bf16 agg strip when p is the last phase."""
                s0, wb = banks[b]
                apb = ps.tile([128, BANK], f32, tag="ps", name=f"agg{l}_{p}_{b}")
                for (w, lo, hi) in pb_wins[(p, b)]:
                    _, _, a, e = windows[w]
                    S = e - a
                    g_sb, slot0 = chunk_of[w]
                    for j in range(TPW):
                        pp = j // (TPW // 2)
                        lt = g_sb[:, slot0 + j, :].rearrange(
                            "p (f two) -> p two f", two=2
                        )[:, pp, :]
                        nc.tensor.matmul(
                            out=apb[:, lo - s0: hi - s0],
                            lhsT=lt,
                            rhs=sel_sb[
                                :,
                                sel_off[w] + j * S + (lo - a):
                                sel_off[w] + j * S + (hi - a),
                            ],
                            start=(j == 0),
                            stop=(j == TPW - 1),
                        )
                if p != CH_ORDER[-1]:
                    if first_phase[b] == p:
                        nc.scalar.activation(
                            out=aggF[:, s0: s0 + wb], in_=apb[:, :wb],
                            func=AF.Copy,
                        )
                    else:
                        nc.vector.tensor_tensor(
                            out=aggF[:, s0: s0 + wb], in0=aggF[:, s0: s0 + wb],
                            in1=apb[:, :wb], op=OP.add,
                        )
                    return None
                aggS = gpool.tile([128, BANK], bf16, tag="aggS", name=f"as{l}{b}")
                if first_phase[b] == CH_ORDER[-1]:
                    nc.scalar.activation(
                        out=aggS[:, :wb], in_=apb[:, :wb], func=AF.Copy
                    )
                else:
                    nc.vector.tensor_tensor(
                        out=aggS[:, :wb], in0=aggF[:, s0: s0 + wb],
                        in1=apb[:, :wb], op=OP.add,
                    )
                return aggS

            bank_order = [bb for c in CH_ORDER for bb in chunk_banks[c]]
            for p in CH_ORDER[:-1]:
                chunk_of = emit_gathers(p)
                for b in bank_order:
                    if (p, b) in pb_wins:
                        agg_bank(p, b, chunk_of)
            p_last = CH_ORDER[-1]
            chunk_of2 = emit_gathers(p_last)
            for c in CH_ORDER:
                for b in chunk_banks[c]:
                    aggS = agg_bank(p_last, b, chunk_of2)
                    gru_bank(l, b, aggS)
                if l < L - 1:
                    h8c = xpool.tile(
                        [128, cb[c + 1] - cb[c]], fp8, tag="h8",
                        name=f"h8_{l}_{c}",
                    )
                    norm_phase(chunk_banks[c], h8=h8c, h8_base=cb[c])
                    store_chunk_and_allgather(l + 1, c, h8c)
                else:
                    norm_phase(chunk_banks[c])
                    decode_cols(cb[c], cb[c + 1])

        stack.close()

    nc.compile()
    return nc


# ---------------------------------------------------------------- entry points
def _prep(inputs):
    import ml_dtypes

    x = np.asarray(inputs["x"], np.float32)
    edge_src = np.asarray(inputs["edge_src"], np.int64)
    edge_dst = np.asarray(inputs["edge_dst"], np.int64)
    n_nodes = x.shape[0]
    meta = _plan(edge_src, edge_dst, n_nodes)

    W1 = np.asarray(inputs["W1"], np.float32)
    b1 = np.asarray(inputs["b1"], np.float32)
    W_ih = np.asarray(inputs["W_ih"], np.float32)
    b_ih = np.asarray(inputs["b_ih"], np.float32)
    W_hh = np.asarray(inputs["W_hh"], np.float32)
    b_hh = np.asarray(inputs["b_hh"], np.float32)
    W2 = np.asarray(inputs["W2"], np.float32)
    b2 = np.asarray(inputs["b2"], np.float32)

    brz = np.stack(
        [
            b_ih[0:128] + b_hh[0:128],
            b_ih[128:256] + b_hh[128:256],
            b_ih[256:384],
            b_hh[256:384],
        ],
        axis=1,
    ).astype(np.float32)

    shared = dict(
        w1T=np.ascontiguousarray(W1.T),
        b1=np.ascontiguousarray(b1[:, None]),
        wihT=np.ascontiguousarray(W_ih.T).astype(ml_dtypes.bfloat16),
        whhT=np.ascontiguousarray(W_hh.T).astype(ml_dtypes.bfloat16),
        brz=brz,
        w2T=np.ascontiguousarray(W2.T),
        binrow=np.ascontiguousarray(b_ih[256:384][None, :]).astype(ml_dtypes.bfloat16),
        bhnrow=np.ascontiguousarray(b_hh[256:384][None, :]).astype(ml_dtypes.bfloat16),
        b2=np.ascontiguousarray(b2[None, :]),
    )
    in_maps = []
    for r in range(CORES):
        xr = x[meta["order_per_core"][r]]
        in_maps.append(
            dict(
                xT=np.ascontiguousarray(xr.T),
                idx=meta["idx_maps"][r],
                sel=meta["sel_maps"][r],
                **shared,
            )
        )
    return meta, in_maps


def _assemble(meta, results, n_nodes):
    npc = meta["npc"]
    out = np.empty((n_nodes, 128), np.float32)
    for r in range(CORES):
        out[meta["order_per_core"][r]] = results[r]["out"][:npc]
    return out


@functools.lru_cache(maxsize=1)
def _get_compiled(key):
    meta, in_maps = _PENDING[key]
    nc = _build(meta)
    return nc, meta, in_maps


_PENDING = {}


def kernel(**inputs):
    x = np.asarray(inputs["x"])
    n_nodes = x.shape[0]
    meta, in_maps = _prep(inputs)
    key = hash(
        (
            n_nodes,
            np.asarray(inputs["edge_src"]).tobytes(),
            np.asarray(inputs["edge_dst"]).tobytes(),
        )
    )
    _PENDING[key] = (meta, in_maps)
    nc, meta, _ = _get_compiled(key)

    from concourse.bass_utils import run_bass_kernel_spmd

    trace = bool(int(os.environ.get("KERNEL_TRACE", "0")))
    res = run_bass_kernel_spmd(
        nc, in_maps, core_ids=list(range(CORES)), trace=trace
    )
    kernel.last_results = res
    return _assemble(meta, res.results, n_nodes)


# revision 24
# speedup vs baseline: 1.2960x; 1.0275x over previous
"""DrBCNet GNN message-passing kernel for 8 Trainium2 NeuronCores — v5.

Strategy (dst-sharded, sparse gather + selector matmuls, 3-chunk pipeline):
  - Nodes globally degree-sorted, dealt round-robin to 8 cores (3750 each);
    GRU/norm/decoder node-local in column layout (hT [feat, pos]).
  - Node positions split into 3 chunks A|B|C (bank-aligned). The h row-table
    (fp8 pairs, [30720 rows, 128B]) lives in DRAM (Shared), rebuilt each
    layer by THREE chunk AllGathers (A, B, C) so collectives pipeline with
    the gather stream.
  - Edges are grouped by (src chunk = phase, dst bank): phase-p gathers read
    only the chunk-p row range of the table (chunk-relative indices), so
    they start as soon as AG_p lands while later AGs are still in flight.
    Per (phase, bank) partial aggregates accumulate via selector matmuls in
    PSUM, evicted into an SBUF f32 accumulator; after the last phase the
    bank's aggregate feeds the GRU. GRU/norm/AG(l+1) run chunk-by-chunk so
    AG_A(l+1) fires while chunk B/C of layer l still compute.
  - GRU: bf16 weight lhsT, i+h gate sums in PSUM, biases folded into the
    PSUM-evacuating activations; Sqrt (l2norm) phase-batched per chunk.
"""

import functools
import os

import numpy as np

CORES = 8
H = 128
L = 5
BANK = 512
NORM_EPS_SQ = 1e-24
WIN_EDGES = int(os.environ.get("WE", "512"))
TPW = WIN_EDGES // 128
HALF_W = WIN_EDGES // 2
# chunk A = leading A_COLS cols; B = [A_COLS, CHB*512); C = rest
A_COLS = int(os.environ.get("ACOLS", "256"))
CHB = int(os.environ.get("CHB", "4"))
CALL_WINDOWS = int(os.environ.get("CW", "2"))  # windows per dma_gather call
# processing order of chunks (chain + phase stream): small A first, big B last
CH_ORDER = [int(x) for x in os.environ.get("CHORD", "0,2,1").split(",")]


# ---------------------------------------------------------------- host planning
def _plan(edge_src, edge_dst, n_nodes):
    npc = n_nodes // CORES
    npc_pad = ((npc + 127) // 128) * 128
    ntiles = npc_pad // 128

    deg = np.bincount(edge_dst, minlength=n_nodes)
    gorder = np.argsort(-deg, kind="stable")
    gpos = np.empty(n_nodes, np.int64)
    gpos[gorder] = np.arange(n_nodes)
    owner = gpos % CORES
    pos = gpos // CORES
    order_per_core = [gorder[r::CORES] for r in range(CORES)]

    # chunks in per-core position space (128-aligned => even sizes)
    cb = [0, A_COLS, CHB * BANK, npc_pad]
    csizes = [cb[1] - cb[0], cb[2] - cb[1], cb[3] - cb[2]]
    # global table row base of each chunk (chunk-major layout)
    cbases = [0, CORES * csizes[0], CORES * (csizes[0] + csizes[1])]
    # bank col ranges: chunk A is one narrow bank, then pad to the B|C
    # boundary, then 512-wide banks
    bank_edges = [0, A_COLS]
    x = A_COLS
    while x < cb[2]:
        step = min(BANK, cb[2] - x)
        x += step
        bank_edges.append(x)
    while x < npc:
        step = min(BANK, npc - x)
        x += step
        bank_edges.append(x)
    banks = [
        (bank_edges[i], min(bank_edges[i + 1], npc) - bank_edges[i])
        for i in range(len(bank_edges) - 1)
    ]
    n_banks = len(banks)
    first_c_bank = next(i for i, (s0, _) in enumerate(banks) if s0 >= cb[2])
    chunk_banks = [
        [0],
        list(range(1, first_c_bank)),
        list(range(first_c_bank, n_banks)),
    ]

    cidx = np.where(pos < cb[1], 0, np.where(pos < cb[2], 1, 2))
    base = np.take(cbases, cidx)
    size = np.take(csizes, cidx)
    start = np.take(np.array(cb[:3]), cidx)
    tpos = base + owner * size + (pos - start)

    srcrow_all = tpos[edge_src]
    src_phase = cidx[edge_src]
    dpos = pos[edge_dst]
    down = owner[edge_dst]

    # per-core per-column counts split by (phase, parity of table row)
    tpar = srcrow_all % 2
    Cnt = np.zeros((3, 2, CORES, npc), np.int64)
    for r in range(CORES):
        m = down == r
        for p in range(3):
            for par in range(2):
                np.add.at(
                    Cnt[p, par, r], dpos[m & (src_phase == p) & (tpar == par)], 1
                )

    # windows per (phase, dst chunk): greedy pack columns across the whole
    # chunk (bank boundaries handled by splitting matmuls in _build).
    # Stream order: phase in CH_ORDER, dst chunk in CH_ORDER.
    windows = []  # list of (phase, chunk, col_a, col_b)
    for p in CH_ORDER:
        for c in CH_ORDER:
            c0 = cb[c]
            c1 = min(cb[c + 1], npc)
            w0 = c0
            rune = np.zeros(CORES, np.int64)
            runo = np.zeros(CORES, np.int64)
            for q in range(c0, c1):
                ce = Cnt[p, 0, :, q]
                co = Cnt[p, 1, :, q]
                if (rune + ce).max() > HALF_W or (runo + co).max() > HALF_W:
                    windows.append((p, c, w0, q))
                    w0 = q
                    rune = ce.copy()
                    runo = co.copy()
                else:
                    rune += ce
                    runo += co
            windows.append((p, c, w0, c1))
    nwin = len(windows)
    win_width = [e - a for (_, _, a, e) in windows]
    sel_off = np.concatenate([[0], np.cumsum([TPW * w for w in win_width])])
    total_sel = int(sel_off[-1])
    total_idx = nwin * WIN_EDGES

    import ml_dtypes

    # per-core idx + selector maps. tiles 0..TPW/2-1 of each window: even
    # parity srcs; rest: odd. idx is the CHUNK-RELATIVE pair row
    # ((tpos - cbases[p]) // 2) so gathers can bind to the chunk row range.
    idx_maps, sel_maps = [], []
    for r in range(CORES):
        eidx = np.nonzero(down == r)[0]
        dp = dpos[eidx]
        sr = srcrow_all[eidx]
        ph = src_phase[eidx]
        key = ph * (npc * 2) + dp * 2 + (sr % 2)
        o = np.argsort(key, kind="stable")
        dp, sr, ph = dp[o], sr[o], ph[o]
        par = sr % 2
        skey = ph * (npc * 2) + dp * 2 + par  # sorted within each phase
        idxs = np.zeros(total_idx, np.int16)
        sel = np.zeros((128, total_sel), np.float32)
        for w, (p, b, a, e) in enumerate(windows):
            S = e - a
            base_i = w * WIN_EDGES
            lo = np.searchsorted(skey, p * (npc * 2) + a * 2)
            hi = np.searchsorted(skey, p * (npc * 2) + e * 2)
            seg = slice(lo, hi)
            pvals = par[seg]
            for pp in (0, 1):
                m = np.nonzero(pvals == pp)[0]
                cnt = len(m)
                assert cnt <= HALF_W, (r, w, pp, cnt)
                slot0 = base_i + pp * HALF_W
                rows_sr = sr[seg][m]
                idxs[slot0: slot0 + cnt] = (
                    (rows_sr - cbases[p]) // 2
                ).astype(np.int16)
                loc = (dp[seg][m] - a).astype(np.int64)
                j = np.arange(cnt) // 128 + (TPW // 2) * pp
                q = np.arange(cnt) % 128
                sel[q, sel_off[w] + j * S + loc] = 1.0
        idx_w = np.zeros((128, total_idx // 16), np.int16)
        wrapped = idxs.reshape(total_idx // 16, 16).T
        for g in range(8):
            idx_w[g * 16: (g + 1) * 16, :] = wrapped
        idx_maps.append(idx_w)
        sel_maps.append(sel.astype(ml_dtypes.float8_e4m3fn))

    # per (phase, bank): windows overlapping the bank with clipped col range
    pb_wins = {}
    phase_wins = {}
    for w, (p, c, a, e) in enumerate(windows):
        for b, (s0, wd) in enumerate(banks):
            lo = max(a, s0)
            hi = min(e, s0 + wd)
            if lo < hi:
                pb_wins.setdefault((p, b), []).append((w, lo, hi))
        if p not in phase_wins:
            phase_wins[p] = [w, w]
        phase_wins[p][1] = w

    return dict(
        npc=npc,
        npc_pad=npc_pad,
        ntiles=ntiles,
        n_banks=n_banks,
        banks=banks,
        cb=cb,
        csizes=csizes,
        cbases=cbases,
        chunk_banks=chunk_banks,
        nwin=nwin,
        windows=windows,
        sel_off=sel_off,
        total_sel=total_sel,
        total_idx=total_idx,
        pb_wins=pb_wins,
        phase_wins=phase_wins,
        order_per_core=order_per_core,
        idx_maps=idx_maps,
        sel_maps=sel_maps,
    )


# ---------------------------------------------------------------- bass program
def _build(meta):
    import concourse.bacc as bacc
    import concourse.mybir as mybir
    import concourse.tile as tile
    from concourse import library_config

    npc = meta["npc"]
    npc_pad = meta["npc_pad"]
    ntiles = meta["ntiles"]
    n_banks = meta["n_banks"]
    banks = meta["banks"]
    cb = meta["cb"]
    csizes = meta["csizes"]
    cbases = meta["cbases"]
    chunk_banks = meta["chunk_banks"]
    windows = meta["windows"]
    sel_off = meta["sel_off"]
    total_sel = meta["total_sel"]
    total_idx = meta["total_idx"]
    pb_wins = meta["pb_wins"]
    phase_wins = meta["phase_wins"]
    n_tbl = CORES * npc_pad
    f32 = mybir.dt.float32
    bf16 = mybir.dt.bfloat16
    i16 = mybir.dt.int16
    fp8 = mybir.dt.float8e4
    AF = mybir.ActivationFunctionType
    OP = mybir.AluOpType

    nc = bacc.Bacc(
        "TRN2",
        target_bir_lowering=False,
        debug=False,
        num_devices=CORES,
        dynamic_dma_scratch_size=int(os.environ.get("SCR", "32768")),
    )

    # I/O
    xT_d = nc.dram_tensor("xT", [3, npc], f32, kind="ExternalInput")
    idx_d = nc.dram_tensor("idx", [128, total_idx // 16], i16, kind="ExternalInput")
    sel_d = nc.dram_tensor("sel", [128, total_sel], fp8, kind="ExternalInput")
    w1T_d = nc.dram_tensor("w1T", [3, 128], f32, kind="ExternalInput")
    b1_d = nc.dram_tensor("b1", [128, 1], f32, kind="ExternalInput")
    wihT_d = nc.dram_tensor("wihT", [128, 3 * H], bf16, kind="ExternalInput")
    whhT_d = nc.dram_tensor("whhT", [128, 3 * H], bf16, kind="ExternalInput")
    brz_d = nc.dram_tensor("brz", [128, 4], f32, kind="ExternalInput")  # br,bz,bin,bhn
    w2T_d = nc.dram_tensor("w2T", [128, 128], f32, kind="ExternalInput")
    b2_d = nc.dram_tensor("b2", [1, 128], f32, kind="ExternalInput")
    binrow_d = nc.dram_tensor("binrow", [1, 128], bf16, kind="ExternalInput")
    bhnrow_d = nc.dram_tensor("bhnrow", [1, 128], bf16, kind="ExternalInput")
    out_d = nc.dram_tensor("out", [npc_pad, 128], f32, kind="ExternalOutput")

    ag_in = [
        nc.dram_tensor(f"agin{l}", [npc_pad // 2, 2, 128], fp8) for l in range(L)
    ]
    tables = [
        nc.dram_tensor(f"table{l}", [n_tbl // 2, 256], fp8, addr_space="Shared")
        for l in range(L)
    ]
    groups = [list(range(CORES))]

    with tile.TileContext(nc) as tc:
        import contextlib

        stack = contextlib.ExitStack()
        nc.gpsimd.load_library(library_config.mlp)
        per = stack.enter_context(tc.tile_pool(name="per", bufs=1))

        def _T(shape, dtype, name=None):
            return per.tile(shape, dtype, name=name, tag=name)

        idx_sb = _T([128, total_idx // 16], i16, name="idx_sb")
        sel_sb = _T([128, total_sel], fp8, name="sel_sb")
        hT = _T([128, npc], f32, name="hT")
        hmaxT = _T([128, npc], f32, name="hmaxT")
        hT16 = _T([128, npc], bf16, name="hT16")
        aggF = _T([128, npc], f32, name="aggF")
        w1T_sb = _T([3, 128], f32, name="w1T_sb")
        b1_sb = _T([128, 1], f32, name="b1_sb")
        wihT_sb = _T([128, 3 * H], bf16, name="wihT_sb")
        whhT_sb = _T([128, 3 * H], bf16, name="whhT_sb")
        brz_sb = _T([128, 4], f32, name="brz_sb")
        w2T_sb = _T([128, 128], f32, name="w2T_sb")
        b2_sb = _T([1, 128], f32, name="b2_sb")
        ones_col = _T([128, 1], f32, name="ones_col")
        ones_row = _T([1, BANK], bf16, name="ones_row")
        binrow = _T([1, 128], bf16, name="binrow")
        bhnrow = _T([1, 128], bf16, name="bhnrow")
        onesk1 = _T([1, 128], f32, name="onesk1")
        eps_sb = _T([1, 1], f32, name="eps_sb")

        gpool = stack.enter_context(
            tc.tile_pool(name="gpool", bufs=int(os.environ.get("GB", "2")))
        )
        epool = stack.enter_context(
            tc.tile_pool(name="epool", bufs=int(os.environ.get("EB", "8")))
        )
        xpool = stack.enter_context(tc.tile_pool(name="xpool", bufs=2))
        tpool = stack.enter_context(
            tc.tile_pool(name="tpool", bufs=int(os.environ.get("TB", "2")))
        )
        ps = stack.enter_context(tc.tile_pool(name="ps", bufs=8, space="PSUM"))

        nc.sync.dma_start(out=aggF[0:3, :], in_=xT_d[:])
        nc.sync.dma_start(out=w1T_sb[:], in_=w1T_d[:])
        nc.sync.dma_start(out=b1_sb[:], in_=b1_d[:])
        nc.sync.dma_start(out=wihT_sb[:], in_=wihT_d[:])
        nc.sync.dma_start(out=whhT_sb[:], in_=whhT_d[:])
        nc.sync.dma_start(out=brz_sb[:], in_=brz_d[:])
        nc.sync.dma_start(out=w2T_sb[:], in_=w2T_d[:])
        nc.sync.dma_start(out=b2_sb[:], in_=b2_d[:])
        nc.vector.memset(eps_sb[:], NORM_EPS_SQ)
        nc.vector.memset(ones_col[:], 1.0)
        nc.vector.memset(ones_row[:], 1.0)
        nc.sync.dma_start(out=binrow[:], in_=binrow_d[:])
        nc.sync.dma_start(out=bhnrow[:], in_=bhnrow_d[:])
        nc.vector.memset(onesk1[:], 1.0)

        def norm_phase(bank_list, h8=None, h8_base=0):
            """l2norm hT strips for several banks; single Sqrt table window.
            If h8 is given, also emit the fp8 column copy (staging input)
            right after each bank's hT update."""
            ns_list = []
            for b in bank_list:
                s0, w = banks[b]
                sq = tpool.tile([128, BANK], f32, tag="sq", name=f"sq{b}")
                nc.vector.tensor_tensor(
                    out=sq[:, :w], in0=hT[:, s0: s0 + w], in1=hT[:, s0: s0 + w],
                    op=OP.mult,
                )
                ns_ps = ps.tile([1, BANK], f32, tag="ps", name=f"ns{b}")
                nc.tensor.matmul(
                    out=ns_ps[:1, :w], lhsT=ones_col[:], rhs=sq[:, :w],
                    start=True, stop=True,
                )
                ns_list.append(ns_ps)
            inv_list = []
            for b, ns_ps in zip(bank_list, ns_list):
                s0, w = banks[b]
                srt = tpool.tile([1, BANK], f32, tag="srt", name=f"srt{b}")
                nc.scalar.activation(
                    out=srt[:1, :w], in_=ns_ps[:1, :w], func=AF.Sqrt,
                    bias=eps_sb[:1, :1],
                )
                inv_t = tpool.tile([1, BANK], f32, tag="inv_t", name=f"inv{b}")
                nc.vector.reciprocal(out=inv_t[:1, :w], in_=srt[:1, :w])
                inv_list.append(inv_t)
            for b, inv_t in zip(bank_list, inv_list):
                s0, w = banks[b]
                bc_ps = ps.tile([128, BANK], f32, tag="ps", name=f"bc{b}")
                nc.tensor.matmul(
                    out=bc_ps[:, :w], lhsT=onesk1[:1, :], rhs=inv_t[:1, :w],
                    start=True, stop=True,
                )
                nc.vector.tensor_tensor(
                    out=hT[:, s0: s0 + w], in0=hT[:, s0: s0 + w],
                    in1=bc_ps[:, :w], op=OP.mult,
                )
                nc.vector.tensor_copy(
                    out=hT16[:, s0: s0 + w], in_=hT[:, s0: s0 + w]
                )
                if h8 is not None:
                    nc.scalar.activation(
                        out=h8[:, s0 - h8_base: s0 - h8_base + w],
                        in_=hT[:, s0: s0 + w], func=AF.Copy,
                    )
                nc.vector.tensor_tensor(
                    out=hmaxT[:, s0: s0 + w], in0=hmaxT[:, s0: s0 + w],
                    in1=hT[:, s0: s0 + w], op=OP.max,
                )

        def gru_bank(l, b, aggS):
            """GRU for bank b; agg strip in SBUF (aggS bf16). Updates hT strip
            (pre-norm). ACT funcs used: Sigmoid/Copy/Tanh only."""
            s0, w = banks[b]
            rz = []
            for g in (0, 1):
                g_ps = ps.tile([128, BANK], f32, tag="ps", name=f"rz{l}{b}{g}")
                nc.tensor.matmul(
                    out=g_ps[:, :w], lhsT=wihT_sb[:, g * H: (g + 1) * H],
                    rhs=aggS[:, :w], start=True, stop=False,
                )
                nc.tensor.matmul(
                    out=g_ps[:, :w], lhsT=whhT_sb[:, g * H: (g + 1) * H],
                    rhs=hT16[:, s0: s0 + w], start=False, stop=True,
                )
                gt = gpool.tile([128, BANK], f32, tag=f"g{g}", name=f"gs{l}{b}{g}")
                nc.scalar.activation(
                    out=gt[:, :w], in_=g_ps[:, :w], func=AF.Sigmoid,
                    bias=brz_sb[:, g: g + 1],
                )
                rz.append(gt)
            r_t, z_t = rz
            in_ps = ps.tile([128, BANK], f32, tag="ps", name=f"in{l}{b}")
            nc.tensor.matmul(
                out=in_ps[:, :w], lhsT=binrow[:1, :], rhs=ones_row[:1, :w],
                start=True, stop=False,
            )
            nc.tensor.matmul(
                out=in_ps[:, :w], lhsT=wihT_sb[:, 2 * H: 3 * H],
                rhs=aggS[:, :w], start=False, stop=True,
            )
            i_n = gpool.tile([128, BANK], f32, tag="gin", name=f"gin{l}{b}")
            nc.scalar.activation(out=i_n[:, :w], in_=in_ps[:, :w], func=AF.Copy)
            hn_ps = ps.tile([128, BANK], f32, tag="ps", name=f"hn{l}{b}")
            nc.tensor.matmul(
                out=hn_ps[:, :w], lhsT=bhnrow[:1, :], rhs=ones_row[:1, :w],
                start=True, stop=False,
            )
            nc.tensor.matmul(
                out=hn_ps[:, :w], lhsT=whhT_sb[:, 2 * H: 3 * H],
                rhs=hT16[:, s0: s0 + w], start=False, stop=True,
            )
            h_n = gpool.tile([128, BANK], f32, tag="ghn", name=f"ghn{l}{b}")
            nc.scalar.activation(out=h_n[:, :w], in_=hn_ps[:, :w], func=AF.Copy)
            n_t = tpool.tile([128, BANK], f32, tag="n_t", name=f"n{l}{b}")
            nc.vector.tensor_tensor(
                out=n_t[:, :w], in0=r_t[:, :w], in1=h_n[:, :w], op=OP.mult
            )
            nc.vector.tensor_tensor(
                out=n_t[:, :w], in0=n_t[:, :w], in1=i_n[:, :w], op=OP.add
            )
            nc.scalar.activation(out=n_t[:, :w], in_=n_t[:, :w], func=AF.Tanh)
            d_t = tpool.tile([128, BANK], f32, tag="d_t", name=f"d{l}{b}")
            nc.vector.tensor_tensor(
                out=d_t[:, :w], in0=hT[:, s0: s0 + w], in1=n_t[:, :w],
                op=OP.subtract,
            )
            nc.vector.tensor_tensor(
                out=d_t[:, :w], in0=d_t[:, :w], in1=z_t[:, :w], op=OP.mult
            )
            nc.vector.tensor_tensor(
                out=hT[:, s0: s0 + w], in0=d_t[:, :w], in1=n_t[:, :w], op=OP.add
            )

        def store_chunk_and_allgather(l, c, h8):
            """Stage chunk c (pre-converted fp8 columns in h8) as INTERLEAVED
            pair rows (byte f*2+par) into ag_in[l] via one XBAR DMA
            transpose, then AllGather into tables[l] chunk-c row range."""
            p0, p1 = cb[c], cb[c + 1]
            csz = p1 - p0
            nblk = csz // 256
            wreal = min(npc, p1) - p0
            if wreal < csz:
                nc.vector.memset(h8[:, wreal:], 0.0)
            rows = xpool.tile(
                [128, nblk, 256], fp8, tag="xbuf", name=f"rows{l}_{c}"
            )
            nc.sync.dma_start_transpose(
                out=rows[:, :, :].bitcast(i16),
                in_=h8[:, :].bitcast(i16),
            )
            dst = (
                ag_in[l]
                .ap()[p0 // 2: p1 // 2, :, :]
                .rearrange("(t k) q f -> k t (q f)", k=128)
            )
            nc.sync.dma_start(out=dst, in_=rows[:, :, :])
            gr0 = cbases[c]
            gr1 = cbases[c] + CORES * csizes[c]
            nc.gpsimd.collective_compute(
                "AllGather",
                mybir.AluOpType.bypass,
                replica_groups=groups,
                ins=[ag_in[l].ap()[p0 // 2: p1 // 2, :, :]],
                outs=[tables[l].ap()[gr0 // 2: gr1 // 2, :]],
            )

        def decode_cols(q0, q1):
            """Decoder for position range [q0, q1) (tile-aligned)."""
            for t in range(q0 // 128, (q1 + 127) // 128):
                wt = min(128, npc - t * 128)
                if wt <= 0:
                    break
                o_ps = ps.tile([128, 128], f32, tag="ps", name=f"dec{t}")
                nc.tensor.matmul(
                    out=o_ps[:wt, :], lhsT=onesk1[:1, :wt], rhs=b2_sb[:1, :],
                    start=True, stop=False,
                )
                nc.tensor.matmul(
                    out=o_ps[:wt, :], lhsT=hmaxT[:, t * 128: t * 128 + wt],
                    rhs=w2T_sb[:], start=False, stop=True,
                )
                orow = tpool.tile([128, 128], f32, tag="orow", name=f"or{t}")
                nc.scalar.activation(
                    out=orow[:wt, :], in_=o_ps[:wt, :], func=AF.Copy
                )
                nc.sync.dma_start(
                    out=out_d[t * 128: t * 128 + wt, :], in_=orow[:wt, :]
                )

        # ---------------- encoder
        for b, (s0, w) in enumerate(banks):
            h0_ps = ps.tile([128, BANK], f32, tag="ps", name=f"enc{b}")
            nc.tensor.matmul(
                out=h0_ps[:, :w], lhsT=w1T_sb[:], rhs=aggF[0:3, s0: s0 + w],
                start=True, stop=True,
            )
            nc.scalar.activation(
                out=hT[:, s0: s0 + w], in_=h0_ps[:, :w], func=AF.Relu,
                bias=b1_sb[:, :1],
            )
            nc.vector.memset(hmaxT[:, s0: s0 + w], -1e30)
        for c in CH_ORDER:
            h8c = xpool.tile(
                [128, cb[c + 1] - cb[c]], fp8, tag="h8", name=f"h8e_{c}"
            )
            norm_phase(chunk_banks[c], h8=h8c, h8_base=cb[c])
            store_chunk_and_allgather(0, c, h8c)

        # deferred bulk loads: needed only once the first gathers run
        nc.scalar.dma_start(out=idx_sb[:], in_=idx_d[:])
        nc.scalar.dma_start(out=sel_sb[:], in_=sel_d[:])

        # ---------------- message-passing layers
        for l in range(L):
            first_phase = {}
            for p in CH_ORDER:
                for b in range(n_banks):
                    if (p, b) in pb_wins and b not in first_phase:
                        first_phase[b] = p

            def emit_gathers(p):
                """Emit the phase-p dma_gather calls (Pool queue order =
                gather stream order). Returns window -> (tile, slot)."""
                pw0, pw1 = phase_wins[p]
                pr0 = cbases[p] // 2
                pr1 = (cbases[p] + CORES * csizes[p]) // 2
                src_ap = tables[l].ap()[pr0:pr1, :]
                chunk_of = {}
                for w in range(pw0, pw1 + 1, CALL_WINDOWS):
                    wlast = min(w + CALL_WINDOWS - 1, pw1)
                    nidx = (wlast - w + 1) * WIN_EDGES
                    g_sb = epool.tile(
                        [128, CALL_WINDOWS * TPW, 256], fp8, tag="gbuf",
                        name=f"g{l}_{p}_{w}",
                    )
                    nc.gpsimd.dma_gather(
                        g_sb[:, : nidx // 128, :],
                        src_ap,
                        idx_sb[
                            :, w * WIN_EDGES // 16: (wlast + 1) * WIN_EDGES // 16
                        ],
                        nidx,
                        nidx,
                        256,
                    )
                    for ww in range(w, wlast + 1):
                        chunk_of[ww] = (g_sb, (ww - w) * TPW)
                return chunk_of

            def agg_bank(p, b, chunk_of):
                """Selector matmuls + eviction for (phase p, bank b).
                Returns the bf16 agg strip when p is the last phase."""
                s0, wb = banks[b]
                apb = ps.tile([128, BANK], f32, tag="ps", name=f"agg{l}_{p}_{b}")
                for (w, lo, hi) in pb_wins[(p, b)]:
                    _, _, a, e = windows[w]
                    S = e - a
                    g_sb, slot0 = chunk_of[w]
                    for j in range(TPW):
                        pp = j // (TPW // 2)
                        lt = g_sb[:, slot0 + j, :].rearrange(
                            "p (f two) -> p two f", two=2
                        )[:, pp, :]
                        nc.tensor.matmul(
                            out=apb[:, lo - s0: hi - s0],
                            lhsT=lt,
                            rhs=sel_sb[
                                :,
                                sel_off[w] + j * S + (lo - a):
                                sel_off[w] + j * S + (hi - a),
                            ],
                            start=(j == 0),
                            stop=(j == TPW - 1),
                        )
                if p != CH_ORDER[-1]:
                    if first_phase[b] == p:
                        nc.scalar.activation(
                            out=aggF[:, s0: s0 + wb], in_=apb[:, :wb],
                            func=AF.Copy,
                        )
                    else:
                        nc.vector.tensor_tensor(
                            out=aggF[:, s0: s0 + wb], in0=aggF[:, s0: s0 + wb],
                            in1=apb[:, :wb], op=OP.add,
                        )
                    return None
                aggS = gpool.tile([128, BANK], bf16, tag="aggS", name=f"as{l}{b}")
                if first_phase[b] == CH_ORDER[-1]:
                    nc.scalar.activation(
                        out=aggS[:, :wb], in_=apb[:, :wb], func=AF.Copy
                    )
                else:
                    nc.vector.tensor_tensor(
                        out=aggS[:, :wb], in0=aggF[:, s0: s0 + wb],
                        in1=apb[:, :wb], op=OP.add,
                    )
                return aggS

            bank_order = [bb for c in CH_ORDER for bb in chunk_banks[c]]
            for p in CH_ORDER[:-1]:
                chunk_of = emit_gathers(p)
                for b in bank_order:
                    if (p, b) in pb_wins:
                        agg_bank(p, b, chunk_of)
            p_last = CH_ORDER[-1]
            chunk_of2 = emit_gathers(p_last)
            for c in CH_ORDER:
                for b in chunk_banks[c]:
                    aggS = agg_bank(p_last, b, chunk_of2)
                    gru_bank(l, b, aggS)
                if l < L - 1:
                    h8c = xpool.tile(
                        [128, cb[c + 1] - cb[c]], fp8, tag="h8",
                        name=f"h8_{l}_{c}",
                    )
                    norm_phase(chunk_banks[c], h8=h8c, h8_base=cb[c])
                    store_chunk_and_allgather(l + 1, c, h8c)
                else:
                    norm_phase(chunk_banks[c])
                    decode_cols(cb[c], cb[c + 1])

        stack.close()

    nc.compile()
    return nc


# ---------------------------------------------------------------- entry points
def _prep(inputs):
    import ml_dtypes

    x = np.asarray(inputs["x"], np.float32)
    edge_src = np.asarray(inputs["edge_src"], np.int64)
    edge_dst = np.asarray(inputs["edge_dst"], np.int64)
    n_nodes = x.shape[0]
    meta = _plan(edge_src, edge_dst, n_nodes)

    W1 = np.asarray(inputs["W1"], np.float32)
    b1 = np.asarray(inputs["b1"], np.float32)
    W_ih = np.asarray(inputs["W_ih"], np.float32)
    b_ih = np.asarray(inputs["b_ih"], np.float32)
    W_hh = np.asarray(inputs["W_hh"], np.float32)
    b_hh = np.asarray(inputs["b_hh"], np.float32)
    W2 = np.asarray(inputs["W2"], np.float32)
    b2 = np.asarray(inputs["b2"], np.float32)

    brz = np.stack(
        [
            b_ih[0:128] + b_hh[0:128],
            b_ih[128:256] + b_hh[128:256],
            b_ih[256:384],
            b_hh[256:384],
        ],
        axis=1,
    ).astype(np.float32)

    shared = dict(
        w1T=np.ascontiguousarray(W1.T),
        b1=np.ascontiguousarray(b1[:, None]),
        wihT=np.ascontiguousarray(W_ih.T).astype(ml_dtypes.bfloat16),
        whhT=np.ascontiguousarray(W_hh.T).astype(ml_dtypes.bfloat16),
        brz=brz,
        w2T=np.ascontiguousarray(W2.T),
        binrow=np.ascontiguousarray(b_ih[256:384][None, :]).astype(ml_dtypes.bfloat16),
        bhnrow=np.ascontiguousarray(b_hh[256:384][None, :]).astype(ml_dtypes.bfloat16),
        b2=np.ascontiguousarray(b2[None, :]),
    )
    in_maps = []
    for r in range(CORES):
        xr = x[meta["order_per_core"][r]]
        in_maps.append(
            dict(
                xT=np.ascontiguousarray(xr.T),
                idx=meta["idx_maps"][r],
                sel=meta["sel_maps"][r],
                **shared,
            )
        )
    return meta, in_maps


def _assemble(meta, results, n_nodes):
    npc = meta["npc"]
    out = np.empty((n_nodes, 128), np.float32)
    for r in range(CORES):
        out[meta["order_per_core"][r]] = results[r]["out"][:npc]
    return out


@functools.lru_cache(maxsize=1)
def _get_compiled(key):
    meta, in_maps = _PENDING[key]
    nc = _build(meta)
    return nc, meta, in_maps


_PENDING = {}


def kernel(**inputs):
    x = np.asarray(inputs["x"])
    n_nodes = x.shape[0]
    meta, in_maps = _prep(inputs)
    key = hash(
        (
            n_nodes,
            np.asarray(inputs["edge_src"]).tobytes(),
            np.asarray(inputs["edge_dst"]).tobytes(),
        )
    )
    _PENDING[key] = (meta, in_maps)
    nc, meta, _ = _get_compiled(key)

    from concourse.bass_utils import run_bass_kernel_spmd

    trace = bool(int(os.environ.get("KERNEL_TRACE", "0")))
    res = run_bass_kernel_spmd(
        nc, in_maps, core_ids=list(range(CORES)), trace=trace
    )
    kernel.last_results = res
    return _assemble(meta, res.results, n_nodes)
